# revision 17
# baseline (speedup 1.0000x reference)
"""Max pairwise L2 distance between two embedding sets, on 8 Trainium2 cores.

Problem: l [8192, 64] f32, r [8192, 64] f32 -> scalar f32
    out = sqrt(max_ij ||l_i - r_j||^2)

Strategy (v2: group-bound certificate)
--------------------------------------
The distance matrix has 67M entries.  On TRN2 only VectorE/ScalarE can read
PSUM (1 fp32/lane/cycle), so exhaustive per-pair examination costs ~35us.
Instead each PSUM entry certifies a whole GROUP of r-points via a provable
upper bound.  For a group G with center c, radius rad = max_j ||r_j - c||,
s = max_j ||r_j||^2:

    max_{j in G} ||l_i - r_j||^2 <= lsq_i + s - 2 l_i.c + 2 ||l_i|| rad

which is a single dot product of augmented vectors:
    rg_vec = [-2c (64) | 1 | 2*rad | s - thr]      (stationary, bf16)
    l_vec  = [l_i (64) | lsq_i | ||l_i|| | 1]      (moving, bf16)

1. Host picks a candidate max L over extreme-norm/extreme-projection pairs
   and sets thr = L - delta (delta rigorously bounds bf16 rounding).
2. Host groups r-points by "heat" (estimated max distance to any l, via the
   candidate l-set): hot points get tiny groups (size 1-2, tight bound),
   the quiet bulk gets coarse groups (4-16).  NG total groups.
3. Device: r-group vectors are the stationary operand (NG/8 = blocks of 128
   per core), all 8192 l-columns stream as moving.  ScalarE relu+accum and
   VectorE max-reduce consume PSUM; a positive partial flags (group,
   l-chunk) cells.  l-rows are heat-sorted so flags concentrate in few
   cells.
4. Host rechecks flagged cells exactly (float64) and returns
   sqrt(max(L, flagged maxima)) - an exact fp32 answer for ANY input;
   grouping quality only affects speed.
"""

import numpy as np
import ml_dtypes

N_CORES = 8
N_L, N_R, DIM = 8192, 8192, 64
K_AUG = 128                     # 64 dims + lsq/ln/1 rows + zero pad to 128
                                # (K<128 streams ~2.2x slower on the PE)
NG = 1024                       # total r-groups (multiple of 128*N_CORES)
N_BLOCKS = NG // (128 * N_CORES)
LCOLS = N_L                     # every core streams all l columns
CHUNK = 1024                    # l-cols consumed per PSUM op (legacy path)
MM_FREE = 512                   # moving free dim per matmul (1 PSUM bank)
PSUM_BUFS = 3
# op plan per block: (engine, free-dim) covering LCOLS; fds are multiples of
# 512 and each 4096-window splits ACT|DVE at a bank boundary (PE-W and
# engine-R must never share a PSUM bank).  Sized so ACT/DVE makespans
# balance: ACT (172+fd)/1.2, DVE (120+fd)/0.96.
PLAN = (("A", 2560), ("D", 1536), ("A", 2048), ("D", 2048))
N_OPS = N_BLOCKS * len(PLAN)
N_ACT = N_BLOCKS * sum(1 for e, _ in PLAN if e == "A")
# hottest r-points first: (count, group_size); counts*sizes must sum to N_R
QUOTA = ((400, 1), (8, 2), (56, 4), (176, 8), (384, 16))
BF16 = ml_dtypes.bfloat16

_COMPILED = {}


def _assignment(n_ops=N_OPS, n_act=N_ACT):
    """Bresenham-spread n_act ScalarE ops among n_ops. True = ACT."""
    out = []
    acc = 0
    for _ in range(n_ops):
        acc += n_act
        if acc >= n_ops:
            acc -= n_ops
            out.append(True)
        else:
            out.append(False)
    assert sum(out) == n_act
    return out


def _dedup_ldweights(nc):
    """Drop InstLdweights whose weights match the previous LDW in the same
    basic block (the PE keeps the loaded stationary across matmuls)."""
    removed = 0
    for fn in nc.m.functions:
        for blk in fn.blocks:
            insts = list(blk.instructions)
            last_sig = None
            keep = []
            for inst in insts:
                if type(inst).__name__ == "InstLdweights":
                    si = inst.sync_info
                    clean = si is None or (
                        not list(si.on_wait) and not list(si.on_update))
                    sig = str(inst.ins[-1])
                    if sig == last_sig and clean:
                        removed += 1
                        continue
                    last_sig = sig
                keep.append(inst)
            if len(keep) != len(insts):
                blk.instructions = keep
    return removed


def _build_nc(n_blocks=N_BLOCKS, lcols=LCOLS, chunk=CHUNK, n_act=None,
              mm_free=MM_FREE, psum_bufs=PSUM_BUFS, dyn_loop=False,
              repeats=1, dedup_ldw=True, consumer="mixed", kaug=K_AUG,
              plan=PLAN):
    """Build + compile the per-core SPMD program.

    Inputs : rg_all [K_AUG, n_blocks*128] bf16, l_all [K_AUG, lcols] bf16
             (+ rep_cnt [1,1] i32 when dyn_loop)
    Outputs: dve_part [128, n_dve] f32 (max of bound-thr over chunk)
             act_part [128, n_act] f32 (sum of relu(bound-thr))
    """
    import concourse.tile as tile
    from concourse import bacc, mybir
    from concourse.bass import make_scalar_value, RegisterHandles

    if plan is not None:
        assert sum(fd for _, fd in plan) == lcols
        ops = list(plan) * n_blocks
    else:
        if n_act is None:
            n_act = (n_blocks * (lcols // chunk)) // 2
        aa = _assignment(n_blocks * (lcols // chunk), n_act)
        ops = [("A" if a else "D", chunk) for a in aa]
    n_ops = len(ops)
    if consumer == "none":
        n_act = 0
        n_dve = 1
    else:
        n_act = sum(1 for e, _ in ops if e == "A")
        n_dve = n_ops - n_act
    ngc = n_blocks * 128

    nc = bacc.Bacc("TRN2", target_bir_lowering=False, debug=False,
                   num_devices=N_CORES)
    bf16 = mybir.dt.bfloat16
    f32 = mybir.dt.float32

    rg_in = nc.dram_tensor("rg_all", [kaug, ngc], bf16,
                           kind="ExternalInput").ap()
    l_in = nc.dram_tensor("l_all", [kaug, lcols], bf16,
                          kind="ExternalInput").ap()
    cnt_in = None
    if dyn_loop:
        cnt_in = nc.dram_tensor("rep_cnt", [1, 1], mybir.dt.int32,
                                kind="ExternalInput").ap()
    dve_out = act_out = None
    if n_dve:
        dve_out = nc.dram_tensor("dve_part", [128, n_dve], f32,
                                 kind="ExternalOutput").ap()
    if n_act:
        act_out = nc.dram_tensor("act_part", [128, n_act], f32,
                                 kind="ExternalOutput").ap()

    with tile.TileContext(nc) as tc:
        with (tc.tile_pool(name="io", bufs=1) as io_pool,
              tc.tile_pool(name="psum", bufs=(1 if plan is not None
                                              else psum_bufs),
                           space="PSUM") as psum_pool,
              tc.tile_pool(name="scratch", bufs=1) as scratch_pool):
            # tiny dummy activation first so the ACT table set loads during
            # the DMA prologue instead of before the first real op
            warm = scratch_pool.tile([128, 1], f32)
            nc.vector.memset(warm[:], 0.0)
            nc.scalar.activation(warm[:], warm[:],
                                 mybir.ActivationFunctionType.Relu)

            rg_sb = io_pool.tile([kaug, ngc], bf16)
            nc.sync.dma_start(rg_sb[:], rg_in[:])
            l_sb = io_pool.tile([kaug, lcols], bf16)
            # first chunk lands first so op 0 starts ASAP
            nc.sync.dma_start(l_sb[:, :chunk], l_in[:, :chunk])
            for c0 in range(chunk, lcols, chunk):
                nc.sync.dma_start(l_sb[:, c0:c0 + chunk],
                                  l_in[:, c0:c0 + chunk])

            dve_sb = act_sb = None
            if n_dve:
                dve_sb = io_pool.tile([128, n_dve], f32, name="dve_sb")
            if n_act:
                act_sb = io_pool.tile([128, n_act], f32, name="act_sb")
            if consumer == "none" and dve_sb is not None:
                nc.vector.memset(dve_sb[:], 0.0)

            if plan is not None:
                ps_all = psum_pool.tile([128, 4096], f32, name="ps_all")

            def body():
                dve_slot = 0
                act_slot = 0
                if plan is None:
                    # legacy path: rotating per-op pool tiles
                    op = 0
                    for b in range(n_blocks):
                        stat = rg_sb[:, b * 128:(b + 1) * 128]
                        for ch in range(lcols // chunk):
                            ps = psum_pool.tile([128, chunk], f32)
                            for k in range(chunk // mm_free):
                                ncol = ch * chunk + k * mm_free
                                nc.tensor.matmul(
                                    ps[:, k * mm_free:(k + 1) * mm_free],
                                    stat, l_sb[:, ncol:ncol + mm_free],
                                    start=True, stop=True)
                            if consumer == "none":
                                op += 1
                                continue
                            eng, _ = ops[op]
                            if eng == "A":
                                nc.scalar.activation(
                                    ps[:, :], ps[:, :],
                                    mybir.ActivationFunctionType.Relu,
                                    accum_out=act_sb[:, act_slot:act_slot + 1])
                                act_slot += 1
                            else:
                                nc.vector.tensor_reduce(
                                    dve_sb[:, dve_slot:dve_slot + 1],
                                    ps[:, :], axis=mybir.AxisListType.X,
                                    op=mybir.AluOpType.max)
                                dve_slot += 1
                            op += 1
                    return
                # plan path: one persistent PSUM tile, offsets cycle mod 4096
                for b in range(n_blocks):
                    stat = rg_sb[:, b * 128:(b + 1) * 128]
                    pos = 0
                    for eng, fd in plan:
                        o = pos % 4096
                        assert o + fd <= 4096
                        for k in range(0, fd, mm_free):
                            w = min(mm_free, fd - k)
                            nc.tensor.matmul(
                                ps_all[:, o + k:o + k + w],
                                stat, l_sb[:, pos + k:pos + k + w],
                                start=True, stop=True)
                        if consumer != "none":
                            if eng == "A":
                                nc.scalar.activation(
                                    ps_all[:, o:o + fd], ps_all[:, o:o + fd],
                                    mybir.ActivationFunctionType.Relu,
                                    accum_out=act_sb[:, act_slot:act_slot + 1])
                                act_slot += 1
                            else:
                                nc.vector.tensor_reduce(
                                    dve_sb[:, dve_slot:dve_slot + 1],
                                    ps_all[:, o:o + fd],
                                    axis=mybir.AxisListType.X,
                                    op=mybir.AluOpType.max)
                                dve_slot += 1
                        pos += fd

            if dyn_loop:
                cnt_sb = io_pool.tile([1, 1], mybir.dt.int32)
                nc.sync.dma_start(cnt_sb[:], cnt_in[:])
                regs = []
                for etype in mybir.ALL_ENGINES:
                    eng = nc.engines[etype]
                    reg = eng.alloc_register(f"repcnt_{etype.name}")
                    eng.reg_load(reg, cnt_sb[0:1, 0:1])
                    regs.append(reg)
                end_sv = make_scalar_value(
                    RegisterHandles(regs), min_val=0, max_val=100000)
                with tc.For_i(0, end_sv):
                    for _ in range(repeats):
                        body()
            else:
                for _ in range(repeats):
                    body()

            if dve_out is not None:
                nc.sync.dma_start(dve_out[:], dve_sb[:])
            if act_out is not None:
                nc.sync.dma_start(act_out[:], act_sb[:])

    if dedup_ldw:
        _dedup_ldweights(nc)
    nc.compile()
    return nc


def _get_nc(key=("full", 1)):
    if key not in _COMPILED:
        kind, repeats = key
        _COMPILED[key] = _build_nc(repeats=repeats)
    return _COMPILED[key]


# ---------------------------------------------------------------- host side

def _candidate_set(x64, xn, nrm_top=96, nproj=24, proj_top=8, seed=777):
    """Indices of extreme-norm / extreme-projection points."""
    cs = set(np.argsort(-xn)[:nrm_top].tolist())
    rng = np.random.default_rng(seed)
    U = rng.standard_normal((nproj, DIM))
    U /= np.linalg.norm(U, axis=1, keepdims=True)
    p = x64 @ U.T
    for k in range(nproj):
        cs.update(np.argsort(-p[:, k])[:proj_top].tolist())
        cs.update(np.argsort(p[:, k])[:proj_top].tolist())
    return np.array(sorted(cs))


def _est_heat(targets, cand_pts):
    """max_i d2(cand_i, target_j) for each target row (float64)."""
    tsq = (targets * targets).sum(1)
    csq = (cand_pts * cand_pts).sum(1)
    d2 = csq[:, None] + tsq[None, :] - 2.0 * (cand_pts @ targets.T)
    return d2.max(axis=0)


def _greedy_pair(pts):
    """Greedy min-distance matching of pts [n, d] -> [n//2, 2] local idx."""
    n = pts.shape[0]
    sq = (pts * pts).sum(1)
    d2 = sq[:, None] + sq[None, :] - 2.0 * (pts @ pts.T)
    np.fill_diagonal(d2, np.inf)
    k = min(8, n - 1)
    nbr = np.argpartition(d2, k - 1, axis=1)[:, :k]
    w = np.take_along_axis(d2, nbr, axis=1)
    edges = np.stack([np.repeat(np.arange(n), k), nbr.ravel(), w.ravel()], 1)
    edges = edges[np.argsort(edges[:, 2])]
    matched = np.zeros(n, dtype=bool)
    pairs = []
    for a, b, _ in edges:
        a, b = int(a), int(b)
        if not matched[a] and not matched[b]:
            matched[a] = matched[b] = True
            pairs.append((a, b))
    rest = np.nonzero(~matched)[0]
    while len(rest) > 1:
        subd = d2[np.ix_(rest, rest)]
        order = np.argsort(subd.ravel())
        used = np.zeros(len(rest), dtype=bool)
        for e in order:
            i, j = divmod(int(e), len(rest))
            if i != j and not used[i] and not used[j]:
                used[i] = used[j] = True
                pairs.append((int(rest[i]), int(rest[j])))
        rest = rest[~used]
    return np.array(pairs, dtype=np.int64)


def _cluster_bucket(r, idx, g):
    """Group r[idx] into size-g groups via hierarchical greedy pairing."""
    if g == 1:
        return [np.array([j]) for j in idx]
    cur = [np.array([j]) for j in idx]
    cents = r[idx].copy()
    while len(cur[0]) < g and len(cur) > 1:
        pairs = _greedy_pair(cents)
        newg, newc = [], []
        used = np.zeros(len(cur), dtype=bool)
        for a, b in pairs:
            na, nb = len(cur[a]), len(cur[b])
            newg.append(np.concatenate([cur[a], cur[b]]))
            newc.append((cents[a] * na + cents[b] * nb) / (na + nb))
            used[a] = used[b] = True
        for i in np.nonzero(~used)[0]:
            newg.append(cur[i])
            newc.append(cents[i])
        cur, cents = newg, np.array(newc)
    return cur


def _one_center(mem):
    """mem [ng, g, d] -> approx minimax centers [ng, d]."""
    c = mem.mean(axis=1)
    for t in range(25):
        d = np.sqrt(((mem - c[:, None, :]) ** 2).sum(-1))
        far = np.argmax(d, axis=1)
        fp = mem[np.arange(len(mem)), far]
        c = c + (1.0 / (t + 3)) * (fp - c)
    return c


def _candidate_threshold(l64, r64, lc, rc):
    """Exact (float64) max squared distance over the candidate pair set."""
    A = l64[lc]
    B = r64[rc]
    d2 = ((A * A).sum(1)[:, None] + (B * B).sum(1)[None, :]
          - 2.0 * (A @ B.T))
    return float(d2.max())


def _bf16_up(x):
    """Round x (f64 array) to bf16, forcing result >= x."""
    x = np.asarray(x, dtype=np.float64)
    y = x.astype(np.float32).astype(BF16)
    bad = y.astype(np.float64) < x
    if np.any(bad):
        bits = y.view(np.uint16)
        pos = (bits & 0x8000) == 0
        # next bf16 toward +inf: +1 ulp for positives, -1 for negatives
        # (negative zero / exact-zero handled via the pos mask on bits)
        up_bits = np.where(pos, bits + 1,
                           np.where(bits == 0x8000, np.uint16(0x0001),
                                    bits - 1)).astype(np.uint16)
        up = up_bits.view(BF16)
        y = np.where(bad, up, y)
        assert np.all(y.astype(np.float64) >= x)
    return y


def _prepare_all(l, r):
    """Returns (in_maps per core, meta dict)."""
    l64 = l.astype(np.float64)
    r64 = r.astype(np.float64)
    lsq = (l64 * l64).sum(1)
    rsq = (r64 * r64).sum(1)
    ln = np.sqrt(lsq)
    rn = np.sqrt(rsq)

    lc = _candidate_set(l64, ln)
    rc = _candidate_set(r64, rn)
    L = _candidate_threshold(l64, r64, lc, rc)

    # heat estimates for grouping / sorting (speed only, not correctness)
    m_est = _est_heat(r64, l64[lc])     # per r-point
    mu_est = _est_heat(l64, r64[rc])    # per l-row

    # group r-points: hottest get smallest groups
    order = np.argsort(m_est)[::-1]
    groups = []
    pos = 0
    for cnt, g in QUOTA:
        idx = order[pos:pos + cnt * g]
        pos += cnt * g
        groups.extend(_cluster_bucket(r64, idx, g))
    assert pos == N_R
    gsz = np.array([len(x) for x in groups])
    ng = len(groups)
    assert ng <= NG, (ng, NG)

    # group stats (vectorized per distinct size)
    cs = np.zeros((ng, DIM))
    rads = np.zeros(ng)
    ss = np.zeros(ng)
    for g in np.unique(gsz):
        sel = np.nonzero(gsz == g)[0]
        if g == 1:
            ids = [groups[i][0] for i in sel]
            cs[sel] = r64[ids]
            ss[sel] = rsq[ids]
            continue
        mem = np.stack([r64[groups[i]] for i in sel])
        c = _one_center(mem)
        cs[sel] = c
        rads[sel] = np.sqrt(((mem - c[:, None, :]) ** 2).sum(-1)).max(1)
        ss[sel] = np.stack([rsq[groups[i]] for i in sel]).max(1)

    # rigorous bf16/accum error bound for the cross term -2 l.c
    cn = np.sqrt((cs * cs).sum(1))
    delta = (2.0 ** -8) * 1.05 * (2.0 * ln.max() * max(cn.max(), 1e-9)) + 0.05
    thr = L - delta

    # device tensors.  bound slots rounded UP so device bound >= true bound
    l_aug = np.zeros((K_AUG, N_L), dtype=BF16)
    rg_aug = np.zeros((K_AUG, NG), dtype=BF16)

    # heat-sorted l (hot rows first -> flags concentrate in early chunks)
    lorder = np.argsort(mu_est)[::-1].copy()
    ls = l64[lorder]
    l_aug[:DIM] = ls.T.astype(np.float32).astype(BF16)
    l_aug[64] = _bf16_up(lsq[lorder])
    l_aug[65] = _bf16_up(ln[lorder])
    l_aug[66] = BF16(1.0)

    rg_aug[:DIM, :ng] = (-2.0 * cs.T).astype(np.float32).astype(BF16)
    rg_aug[64, :ng] = BF16(1.0)
    rg_aug[65, :ng] = _bf16_up(2.0 * rads)
    rg_aug[66, :ng] = _bf16_up(ss - thr)
    if ng < NG:  # padding groups: never flag
        rg_aug[66, ng:] = BF16(-1000.0)

    in_maps = [
        {"rg_all": np.ascontiguousarray(
            rg_aug[:, c * N_BLOCKS * 128:(c + 1) * N_BLOCKS * 128]),
         "l_all": np.ascontiguousarray(l_aug)}
        for c in range(N_CORES)
    ]
    meta = dict(groups=groups, gsz=gsz, ng=ng, L=L, thr=thr, delta=delta,
                lorder=lorder, lsq=lsq, rsq=rsq, l64=l64, r64=r64)
    return in_maps, meta


def _run_device(in_maps, nc=None):
    from concourse.bass_utils import run_bass_kernel_spmd
    if nc is None:
        nc = _get_nc()
    res = run_bass_kernel_spmd(nc, in_maps, core_ids=list(range(N_CORES)))
    return res.results


def _decode_and_recheck(results, meta):
    """Exact float64 recheck of flagged (group, l-chunk) cells."""
    groups = meta["groups"]
    lorder = meta["lorder"]
    lsq, rsq = meta["lsq"], meta["rsq"]
    l64, r64 = meta["l64"], meta["r64"]
    best = meta["L"]
    ng = meta["ng"]
    # op list: (engine, block, l-range)
    op_list = []
    for b in range(N_BLOCKS):
        pos = 0
        for eng, fd in PLAN:
            op_list.append((eng, b, pos, fd))
            pos += fd

    for core in range(N_CORES):
        dve = results[core].get("dve_part")
        act = results[core].get("act_part")
        dve_slot = act_slot = 0
        for eng, b, pos, fd in op_list:
            if eng == "A":
                part = act[:, act_slot]
                act_slot += 1
            else:
                part = dve[:, dve_slot]
                dve_slot += 1
            lanes = np.nonzero(part > 0.0)[0]
            if lanes.size == 0:
                continue
            rows = lorder[pos:pos + fd]
            members = []
            for p in lanes:
                gid = (core * N_BLOCKS + b) * 128 + int(p)
                if gid < ng:
                    members.append(groups[gid])
            if not members:
                continue
            mem = np.concatenate(members)
            d2 = (lsq[rows][:, None] + rsq[mem][None, :]
                  - 2.0 * (l64[rows] @ r64[mem].T))
            best = max(best, float(d2.max()))
    return best


def kernel(l_dfa_embeddings, r_dfa_embeddings):
    l = np.asarray(l_dfa_embeddings, dtype=np.float32)
    r = np.asarray(r_dfa_embeddings, dtype=np.float32)
    assert l.shape == (N_L, DIM) and r.shape == (N_R, DIM)

    in_maps, meta = _prepare_all(l, r)
    results = _run_device(in_maps)
    best = _decode_and_recheck(results, meta)
    return np.float32(np.sqrt(max(best, 0.0)))


# revision 21
# speedup vs baseline: 1.5042x; 1.5042x over previous
"""Max pairwise L2 distance between two embedding sets, on 8 Trainium2 cores.

Problem: l [8192, 64] f32, r [8192, 64] f32 -> scalar f32
    out = sqrt(max_ij ||l_i - r_j||^2)

Strategy (v2: group-bound certificate)
--------------------------------------
The distance matrix has 67M entries.  On TRN2 only VectorE/ScalarE can read
PSUM (1 fp32/lane/cycle), so exhaustive per-pair examination costs ~35us.
Instead each PSUM entry certifies a whole GROUP of r-points via a provable
upper bound.  For a group G with center c, radius rad = max_j ||r_j - c||,
s = max_j ||r_j||^2:

    max_{j in G} ||l_i - r_j||^2 <= lsq_i + s - 2 l_i.c + 2 ||l_i|| rad

which is a single dot product of augmented vectors:
    rg_vec = [-2c (64) | 1 | 2*rad | s - thr]      (stationary, bf16)
    l_vec  = [l_i (64) | lsq_i | ||l_i|| | 1]      (moving, bf16)

1. Host picks a candidate max L over extreme-norm/extreme-projection pairs
   and sets thr = L - delta (delta rigorously bounds bf16 rounding).
2. Host groups r-points by "heat" (estimated max distance to any l, via the
   candidate l-set): hot points get tiny groups (size 1-2, tight bound),
   the quiet bulk gets coarse groups (4-16).  NG total groups.
3. Device: r-group vectors are the stationary operand (NG/8 = blocks of 128
   per core), all 8192 l-columns stream as moving.  ScalarE relu+accum and
   VectorE max-reduce consume PSUM; a positive partial flags (group,
   l-chunk) cells.  l-rows are heat-sorted so flags concentrate in few
   cells.
4. Host rechecks flagged cells exactly (float64) and returns
   sqrt(max(L, flagged maxima)) - an exact fp32 answer for ANY input;
   grouping quality only affects speed.
"""

import numpy as np
import ml_dtypes

N_CORES = 8
N_L, N_R, DIM = 8192, 8192, 64
K_AUG = 128                     # 64 dims + lsq/ln/1 rows + zero pad to 128
                                # (K<128 streams ~2.2x slower on the PE)
NG = 1024                       # total r-groups (multiple of 128*N_CORES)
N_BLOCKS = NG // (128 * N_CORES)
LCOLS = N_L                     # every core streams all l columns
CHUNK = 1024                    # l-cols consumed per PSUM op
MM_FREE = 512                   # moving free dim per matmul (1 PSUM bank)
PSUM_BUFS = 4                   # rotating pool tiles (4 x 1024 = full PSUM)
# PLAN: optional explicit (engine, fd) op list per block for the
# persistent-tile path.  Measured slower than pool rotation (overlap-dep
# chains), so production uses PLAN=None -> uniform CHUNK ops, N_ACT of
# them on ScalarE (rest VectorE), Bresenham-interleaved.
PLAN = None
N_OPS = N_BLOCKS * (LCOLS // CHUNK)
N_ACT = 4
# hottest r-points first: (count, group_size); counts*sizes must sum to N_R
QUOTA = ((400, 1), (8, 2), (56, 4), (176, 8), (384, 16))
BF16 = ml_dtypes.bfloat16

_COMPILED = {}


def _op_list():
    """Production op list per block: [(engine, fd), ...] covering LCOLS."""
    if PLAN is not None:
        return list(PLAN)
    aa = _assignment(LCOLS // CHUNK, N_ACT)
    return [("A" if a else "D", CHUNK) for a in aa]


def _assignment(n_ops=N_OPS, n_act=N_ACT):
    """Bresenham-spread n_act ScalarE ops among n_ops. True = ACT."""
    out = []
    acc = 0
    for _ in range(n_ops):
        acc += n_act
        if acc >= n_ops:
            acc -= n_ops
            out.append(True)
        else:
            out.append(False)
    assert sum(out) == n_act
    return out


def _dedup_ldweights(nc):
    """Drop InstLdweights whose weights match the previous LDW in the same
    basic block (the PE keeps the loaded stationary across matmuls)."""
    removed = 0
    for fn in nc.m.functions:
        for blk in fn.blocks:
            insts = list(blk.instructions)
            last_sig = None
            keep = []
            for inst in insts:
                if type(inst).__name__ == "InstLdweights":
                    si = inst.sync_info
                    clean = si is None or (
                        not list(si.on_wait) and not list(si.on_update))
                    sig = str(inst.ins[-1])
                    if sig == last_sig and clean:
                        removed += 1
                        continue
                    last_sig = sig
                keep.append(inst)
            if len(keep) != len(insts):
                blk.instructions = keep
    return removed


def _build_nc(n_blocks=N_BLOCKS, lcols=LCOLS, chunk=CHUNK, n_act=None,
              mm_free=MM_FREE, psum_bufs=PSUM_BUFS, dyn_loop=False,
              repeats=1, dedup_ldw=True, consumer="mixed", kaug=K_AUG,
              plan=PLAN):
    """Build + compile the per-core SPMD program.

    Inputs : rg_all [K_AUG, n_blocks*128] bf16, l_all [K_AUG, lcols] bf16
             (+ rep_cnt [1,1] i32 when dyn_loop)
    Outputs: dve_part [128, n_dve] f32 (max of bound-thr over chunk)
             act_part [128, n_act] f32 (sum of relu(bound-thr))
    """
    import concourse.tile as tile
    from concourse import bacc, mybir
    from concourse.bass import make_scalar_value, RegisterHandles

    if plan is not None:
        assert sum(fd for _, fd in plan) == lcols
        ops = list(plan) * n_blocks
    else:
        if n_act is None:
            n_act = N_ACT * (n_blocks * (lcols // chunk)) // N_OPS
        aa = _assignment(n_blocks * (lcols // chunk), n_act)
        ops = [("A" if a else "D", chunk) for a in aa]
    n_ops = len(ops)
    if consumer == "none":
        n_act = 0
        n_dve = 1
    else:
        n_act = sum(1 for e, _ in ops if e == "A")
        n_dve = n_ops - n_act
    ngc = n_blocks * 128

    nc = bacc.Bacc("TRN2", target_bir_lowering=False, debug=False,
                   num_devices=N_CORES)
    bf16 = mybir.dt.bfloat16
    f32 = mybir.dt.float32

    rg_in = nc.dram_tensor("rg_all", [kaug, ngc], bf16,
                           kind="ExternalInput").ap()
    l_in = nc.dram_tensor("l_all", [kaug, lcols], bf16,
                          kind="ExternalInput").ap()
    cnt_in = None
    if dyn_loop:
        cnt_in = nc.dram_tensor("rep_cnt", [1, 1], mybir.dt.int32,
                                kind="ExternalInput").ap()
    dve_out = act_out = None
    if n_dve:
        dve_out = nc.dram_tensor("dve_part", [128, n_dve], f32,
                                 kind="ExternalOutput").ap()
    if n_act:
        act_out = nc.dram_tensor("act_part", [128, n_act], f32,
                                 kind="ExternalOutput").ap()

    with tile.TileContext(nc) as tc:
        with (tc.tile_pool(name="io", bufs=1) as io_pool,
              tc.tile_pool(name="psum", bufs=(1 if plan is not None
                                              else psum_bufs),
                           space="PSUM") as psum_pool,
              tc.tile_pool(name="scratch", bufs=1) as scratch_pool):
            # tiny dummy activation first so the ACT table set loads during
            # the DMA prologue instead of before the first real op
            warm = scratch_pool.tile([128, 1], f32)
            nc.vector.memset(warm[:], 0.0)
            nc.scalar.activation(warm[:], warm[:],
                                 mybir.ActivationFunctionType.Relu)

            rg_sb = io_pool.tile([kaug, ngc], bf16)
            nc.sync.dma_start(rg_sb[:], rg_in[:])
            l_sb = io_pool.tile([kaug, lcols], bf16)
            # first chunk lands first so op 0 starts ASAP
            nc.sync.dma_start(l_sb[:, :chunk], l_in[:, :chunk])
            for c0 in range(chunk, lcols, chunk):
                nc.sync.dma_start(l_sb[:, c0:c0 + chunk],
                                  l_in[:, c0:c0 + chunk])

            dve_sb = act_sb = None
            if n_dve:
                dve_sb = io_pool.tile([128, n_dve], f32, name="dve_sb")
            if n_act:
                act_sb = io_pool.tile([128, n_act], f32, name="act_sb")
            if consumer == "none" and dve_sb is not None:
                nc.vector.memset(dve_sb[:], 0.0)

            if plan is not None:
                ps_all = psum_pool.tile([128, 4096], f32, name="ps_all")

            def body():
                dve_slot = 0
                act_slot = 0
                if plan is None:
                    # legacy path: rotating per-op pool tiles
                    op = 0
                    for b in range(n_blocks):
                        stat = rg_sb[:, b * 128:(b + 1) * 128]
                        for ch in range(lcols // chunk):
                            ps = psum_pool.tile([128, chunk], f32)
                            for k in range(chunk // mm_free):
                                ncol = ch * chunk + k * mm_free
                                nc.tensor.matmul(
                                    ps[:, k * mm_free:(k + 1) * mm_free],
                                    stat, l_sb[:, ncol:ncol + mm_free],
                                    start=True, stop=True)
                            if consumer == "none":
                                op += 1
                                continue
                            eng, _ = ops[op]
                            if eng == "A":
                                nc.scalar.activation(
                                    ps[:, :], ps[:, :],
                                    mybir.ActivationFunctionType.Relu,
                                    accum_out=act_sb[:, act_slot:act_slot + 1])
                                act_slot += 1
                            else:
                                nc.vector.tensor_reduce(
                                    dve_sb[:, dve_slot:dve_slot + 1],
                                    ps[:, :], axis=mybir.AxisListType.X,
                                    op=mybir.AluOpType.max)
                                dve_slot += 1
                            op += 1
                    return
                # plan path: one persistent PSUM tile, offsets cycle mod 4096
                for b in range(n_blocks):
                    stat = rg_sb[:, b * 128:(b + 1) * 128]
                    pos = 0
                    for eng, fd in plan:
                        o = pos % 4096
                        assert o + fd <= 4096
                        for k in range(0, fd, mm_free):
                            w = min(mm_free, fd - k)
                            nc.tensor.matmul(
                                ps_all[:, o + k:o + k + w],
                                stat, l_sb[:, pos + k:pos + k + w],
                                start=True, stop=True)
                        if consumer != "none":
                            if eng == "A":
                                nc.scalar.activation(
                                    ps_all[:, o:o + fd], ps_all[:, o:o + fd],
                                    mybir.ActivationFunctionType.Relu,
                                    accum_out=act_sb[:, act_slot:act_slot + 1])
                                act_slot += 1
                            else:
                                nc.vector.tensor_reduce(
                                    dve_sb[:, dve_slot:dve_slot + 1],
                                    ps_all[:, o:o + fd],
                                    axis=mybir.AxisListType.X,
                                    op=mybir.AluOpType.max)
                                dve_slot += 1
                        pos += fd

            if dyn_loop:
                cnt_sb = io_pool.tile([1, 1], mybir.dt.int32)
                nc.sync.dma_start(cnt_sb[:], cnt_in[:])
                regs = []
                for etype in mybir.ALL_ENGINES:
                    eng = nc.engines[etype]
                    reg = eng.alloc_register(f"repcnt_{etype.name}")
                    eng.reg_load(reg, cnt_sb[0:1, 0:1])
                    regs.append(reg)
                end_sv = make_scalar_value(
                    RegisterHandles(regs), min_val=0, max_val=100000)
                with tc.For_i(0, end_sv):
                    for _ in range(repeats):
                        body()
            else:
                for _ in range(repeats):
                    body()

            if dve_out is not None:
                nc.sync.dma_start(dve_out[:], dve_sb[:])
            if act_out is not None:
                nc.sync.dma_start(act_out[:], act_sb[:])

    if dedup_ldw:
        _dedup_ldweights(nc)
    nc.compile()
    return nc


def _get_nc(key=("full", 1)):
    if key not in _COMPILED:
        kind, repeats = key
        _COMPILED[key] = _build_nc(repeats=repeats)
    return _COMPILED[key]


# ---------------------------------------------------------------- host side

def _candidate_set(x64, xn, nrm_top=96, nproj=24, proj_top=8, seed=777):
    """Indices of extreme-norm / extreme-projection points."""
    cs = set(np.argsort(-xn)[:nrm_top].tolist())
    rng = np.random.default_rng(seed)
    U = rng.standard_normal((nproj, DIM))
    U /= np.linalg.norm(U, axis=1, keepdims=True)
    p = x64 @ U.T
    for k in range(nproj):
        cs.update(np.argsort(-p[:, k])[:proj_top].tolist())
        cs.update(np.argsort(p[:, k])[:proj_top].tolist())
    return np.array(sorted(cs))


def _est_heat(targets, cand_pts):
    """max_i d2(cand_i, target_j) for each target row (float64)."""
    tsq = (targets * targets).sum(1)
    csq = (cand_pts * cand_pts).sum(1)
    d2 = csq[:, None] + tsq[None, :] - 2.0 * (cand_pts @ targets.T)
    return d2.max(axis=0)


def _greedy_pair(pts):
    """Greedy min-distance matching of pts [n, d] -> [n//2, 2] local idx."""
    n = pts.shape[0]
    sq = (pts * pts).sum(1)
    d2 = sq[:, None] + sq[None, :] - 2.0 * (pts @ pts.T)
    np.fill_diagonal(d2, np.inf)
    k = min(8, n - 1)
    nbr = np.argpartition(d2, k - 1, axis=1)[:, :k]
    w = np.take_along_axis(d2, nbr, axis=1)
    edges = np.stack([np.repeat(np.arange(n), k), nbr.ravel(), w.ravel()], 1)
    edges = edges[np.argsort(edges[:, 2])]
    matched = np.zeros(n, dtype=bool)
    pairs = []
    for a, b, _ in edges:
        a, b = int(a), int(b)
        if not matched[a] and not matched[b]:
            matched[a] = matched[b] = True
            pairs.append((a, b))
    rest = np.nonzero(~matched)[0]
    while len(rest) > 1:
        subd = d2[np.ix_(rest, rest)]
        order = np.argsort(subd.ravel())
        used = np.zeros(len(rest), dtype=bool)
        for e in order:
            i, j = divmod(int(e), len(rest))
            if i != j and not used[i] and not used[j]:
                used[i] = used[j] = True
                pairs.append((int(rest[i]), int(rest[j])))
        rest = rest[~used]
    return np.array(pairs, dtype=np.int64)


def _cluster_bucket(r, idx, g):
    """Group r[idx] into size-g groups via hierarchical greedy pairing."""
    if g == 1:
        return [np.array([j]) for j in idx]
    cur = [np.array([j]) for j in idx]
    cents = r[idx].copy()
    while len(cur[0]) < g and len(cur) > 1:
        pairs = _greedy_pair(cents)
        newg, newc = [], []
        used = np.zeros(len(cur), dtype=bool)
        for a, b in pairs:
            na, nb = len(cur[a]), len(cur[b])
            newg.append(np.concatenate([cur[a], cur[b]]))
            newc.append((cents[a] * na + cents[b] * nb) / (na + nb))
            used[a] = used[b] = True
        for i in np.nonzero(~used)[0]:
            newg.append(cur[i])
            newc.append(cents[i])
        cur, cents = newg, np.array(newc)
    return cur


def _one_center(mem):
    """mem [ng, g, d] -> approx minimax centers [ng, d]."""
    c = mem.mean(axis=1)
    for t in range(25):
        d = np.sqrt(((mem - c[:, None, :]) ** 2).sum(-1))
        far = np.argmax(d, axis=1)
        fp = mem[np.arange(len(mem)), far]
        c = c + (1.0 / (t + 3)) * (fp - c)
    return c


def _candidate_threshold(l64, r64, lc, rc):
    """Exact (float64) max squared distance over the candidate pair set."""
    A = l64[lc]
    B = r64[rc]
    d2 = ((A * A).sum(1)[:, None] + (B * B).sum(1)[None, :]
          - 2.0 * (A @ B.T))
    return float(d2.max())


def _bf16_up(x):
    """Round x (f64 array) to bf16, forcing result >= x."""
    x = np.asarray(x, dtype=np.float64)
    y = x.astype(np.float32).astype(BF16)
    bad = y.astype(np.float64) < x
    if np.any(bad):
        bits = y.view(np.uint16)
        pos = (bits & 0x8000) == 0
        # next bf16 toward +inf: +1 ulp for positives, -1 for negatives
        # (negative zero / exact-zero handled via the pos mask on bits)
        up_bits = np.where(pos, bits + 1,
                           np.where(bits == 0x8000, np.uint16(0x0001),
                                    bits - 1)).astype(np.uint16)
        up = up_bits.view(BF16)
        y = np.where(bad, up, y)
        assert np.all(y.astype(np.float64) >= x)
    return y


def _prepare_all(l, r):
    """Returns (in_maps per core, meta dict)."""
    l64 = l.astype(np.float64)
    r64 = r.astype(np.float64)
    lsq = (l64 * l64).sum(1)
    rsq = (r64 * r64).sum(1)
    ln = np.sqrt(lsq)
    rn = np.sqrt(rsq)

    lc = _candidate_set(l64, ln)
    rc = _candidate_set(r64, rn)
    L = _candidate_threshold(l64, r64, lc, rc)

    # heat estimates for grouping / sorting (speed only, not correctness)
    m_est = _est_heat(r64, l64[lc])     # per r-point
    mu_est = _est_heat(l64, r64[rc])    # per l-row

    # group r-points: hottest get smallest groups
    order = np.argsort(m_est)[::-1]
    groups = []
    pos = 0
    for cnt, g in QUOTA:
        idx = order[pos:pos + cnt * g]
        pos += cnt * g
        groups.extend(_cluster_bucket(r64, idx, g))
    assert pos == N_R
    gsz = np.array([len(x) for x in groups])
    ng = len(groups)
    assert ng <= NG, (ng, NG)

    # group stats (vectorized per distinct size)
    cs = np.zeros((ng, DIM))
    rads = np.zeros(ng)
    ss = np.zeros(ng)
    for g in np.unique(gsz):
        sel = np.nonzero(gsz == g)[0]
        if g == 1:
            ids = [groups[i][0] for i in sel]
            cs[sel] = r64[ids]
            ss[sel] = rsq[ids]
            continue
        mem = np.stack([r64[groups[i]] for i in sel])
        c = _one_center(mem)
        cs[sel] = c
        rads[sel] = np.sqrt(((mem - c[:, None, :]) ** 2).sum(-1)).max(1)
        ss[sel] = np.stack([rsq[groups[i]] for i in sel]).max(1)

    # rigorous bf16/accum error bound for the cross term -2 l.c
    cn = np.sqrt((cs * cs).sum(1))
    delta = (2.0 ** -8) * 1.05 * (2.0 * ln.max() * max(cn.max(), 1e-9)) + 0.05
    thr = L - delta

    # device tensors.  bound slots rounded UP so device bound >= true bound
    l_aug = np.zeros((K_AUG, N_L), dtype=BF16)
    rg_aug = np.zeros((K_AUG, NG), dtype=BF16)

    # heat-sorted l (hot rows first -> flags concentrate in early chunks)
    lorder = np.argsort(mu_est)[::-1].copy()
    ls = l64[lorder]
    l_aug[:DIM] = ls.T.astype(np.float32).astype(BF16)
    l_aug[64] = _bf16_up(lsq[lorder])
    l_aug[65] = _bf16_up(ln[lorder])
    l_aug[66] = BF16(1.0)

    rg_aug[:DIM, :ng] = (-2.0 * cs.T).astype(np.float32).astype(BF16)
    rg_aug[64, :ng] = BF16(1.0)
    rg_aug[65, :ng] = _bf16_up(2.0 * rads)
    rg_aug[66, :ng] = _bf16_up(ss - thr)
    if ng < NG:  # padding groups: never flag
        rg_aug[66, ng:] = BF16(-1000.0)

    in_maps = [
        {"rg_all": np.ascontiguousarray(
            rg_aug[:, c * N_BLOCKS * 128:(c + 1) * N_BLOCKS * 128]),
         "l_all": np.ascontiguousarray(l_aug)}
        for c in range(N_CORES)
    ]
    meta = dict(groups=groups, gsz=gsz, ng=ng, L=L, thr=thr, delta=delta,
                lorder=lorder, lsq=lsq, rsq=rsq, l64=l64, r64=r64)
    return in_maps, meta


def _run_device(in_maps, nc=None):
    from concourse.bass_utils import run_bass_kernel_spmd
    if nc is None:
        nc = _get_nc()
    res = run_bass_kernel_spmd(nc, in_maps, core_ids=list(range(N_CORES)))
    return res.results


def _decode_and_recheck(results, meta):
    """Exact float64 recheck of flagged (group, l-chunk) cells."""
    groups = meta["groups"]
    lorder = meta["lorder"]
    lsq, rsq = meta["lsq"], meta["rsq"]
    l64, r64 = meta["l64"], meta["r64"]
    best = meta["L"]
    ng = meta["ng"]
    # op list: (engine, block, l-range)
    op_list = []
    for b in range(N_BLOCKS):
        pos = 0
        for eng, fd in _op_list():
            op_list.append((eng, b, pos, fd))
            pos += fd

    for core in range(N_CORES):
        dve = results[core].get("dve_part")
        act = results[core].get("act_part")
        dve_slot = act_slot = 0
        for eng, b, pos, fd in op_list:
            if eng == "A":
                part = act[:, act_slot]
                act_slot += 1
            else:
                part = dve[:, dve_slot]
                dve_slot += 1
            lanes = np.nonzero(part > 0.0)[0]
            if lanes.size == 0:
                continue
            rows = lorder[pos:pos + fd]
            members = []
            for p in lanes:
                gid = (core * N_BLOCKS + b) * 128 + int(p)
                if gid < ng:
                    members.append(groups[gid])
            if not members:
                continue
            mem = np.concatenate(members)
            d2 = (lsq[rows][:, None] + rsq[mem][None, :]
                  - 2.0 * (l64[rows] @ r64[mem].T))
            best = max(best, float(d2.max()))
    return best


def kernel(l_dfa_embeddings, r_dfa_embeddings):
    l = np.asarray(l_dfa_embeddings, dtype=np.float32)
    r = np.asarray(r_dfa_embeddings, dtype=np.float32)
    assert l.shape == (N_L, DIM) and r.shape == (N_R, DIM)

    in_maps, meta = _prepare_all(l, r)
    results = _run_device(in_maps)
    best = _decode_and_recheck(results, meta)
    return np.float32(np.sqrt(max(best, 0.0)))


# revision 22
# speedup vs baseline: 2.0796x; 1.3825x over previous
"""Max pairwise L2 distance between two embedding sets, on 8 Trainium2 cores.

Problem: l [8192, 64] f32, r [8192, 64] f32 -> scalar f32
    out = sqrt(max_ij ||l_i - r_j||^2)

Strategy (v2: group-bound certificate)
--------------------------------------
The distance matrix has 67M entries.  On TRN2 only VectorE/ScalarE can read
PSUM (1 fp32/lane/cycle), so exhaustive per-pair examination costs ~35us.
Instead each PSUM entry certifies a whole GROUP of r-points via a provable
upper bound.  For a group G with center c, radius rad = max_j ||r_j - c||,
s = max_j ||r_j||^2:

    max_{j in G} ||l_i - r_j||^2 <= lsq_i + s - 2 l_i.c + 2 ||l_i|| rad

which is a single dot product of augmented vectors:
    rg_vec = [-2c (64) | 1 | 2*rad | s - thr]      (stationary, bf16)
    l_vec  = [l_i (64) | lsq_i | ||l_i|| | 1]      (moving, bf16)

1. Host picks a candidate max L over extreme-norm/extreme-projection pairs
   and sets thr = L - delta (delta rigorously bounds bf16 rounding).
2. Host groups r-points by "heat" (estimated max distance to any l, via the
   candidate l-set): hot points get tiny groups (size 1-2, tight bound),
   the quiet bulk gets coarse groups (4-16).  NG total groups.
3. Device: r-group vectors are the stationary operand (NG/8 = blocks of 128
   per core), all 8192 l-columns stream as moving.  ScalarE relu+accum and
   VectorE max-reduce consume PSUM; a positive partial flags (group,
   l-chunk) cells.  l-rows are heat-sorted so flags concentrate in few
   cells.
4. Host rechecks flagged cells exactly (float64) and returns
   sqrt(max(L, flagged maxima)) - an exact fp32 answer for ANY input;
   grouping quality only affects speed.

Measured (8-core SPMD, per-pass device time): ~4.4-5.3us vs the 50-53us
v1 exhaustive kernel.  Key HW facts: the PE streams bf16 matmuls ~2.2x
slower when the contraction dim K < 128, so the augmented vectors are
zero-padded to K=128 (2.6us for 8192 moving cols at the production
roofline); PSUM consumption is the bottleneck (ScalarE (172+FD)/1.2GHz,
VectorE (120+FD)/0.96GHz per op), balanced 4 ACT + 4 DVE ops of FD=1024
over a 4-deep rotating PSUM pool.  Uneven ACT/DVE free dims via a single
persistent PSUM tile measured slower (overlap-dep chains serialize), and
PE-W + engine-R must never share a PSUM bank (fatal HW hazard), which
quantizes op regions to 512-f32 banks.
"""

import numpy as np
import ml_dtypes

N_CORES = 8
N_L, N_R, DIM = 8192, 8192, 64
K_AUG = 128                     # 64 dims + lsq/ln/1 rows + zero pad to 128
                                # (K<128 streams ~2.2x slower on the PE)
NG = 1024                       # total r-groups (multiple of 128*N_CORES)
N_BLOCKS = NG // (128 * N_CORES)
LCOLS = N_L                     # every core streams all l columns
CHUNK = 1024                    # l-cols consumed per PSUM op
MM_FREE = 512                   # moving free dim per matmul (1 PSUM bank)
PSUM_BUFS = 4                   # rotating pool tiles (4 x 1024 = full PSUM)
# PLAN: optional explicit (engine, fd) op list per block for the
# persistent-tile path.  Measured slower than pool rotation (overlap-dep
# chains), so production uses PLAN=None -> uniform CHUNK ops, N_ACT of
# them on ScalarE (rest VectorE), Bresenham-interleaved.
PLAN = None
N_OPS = N_BLOCKS * (LCOLS // CHUNK)
N_ACT = 4
# hottest r-points first: (count, group_size); counts*sizes must sum to N_R
QUOTA = ((400, 1), (8, 2), (56, 4), (176, 8), (384, 16))
BF16 = ml_dtypes.bfloat16

_COMPILED = {}


def _op_list():
    """Production op list per block: [(engine, fd), ...] covering LCOLS."""
    if PLAN is not None:
        return list(PLAN)
    aa = _assignment(LCOLS // CHUNK, N_ACT)
    return [("A" if a else "D", CHUNK) for a in aa]


def _assignment(n_ops=N_OPS, n_act=N_ACT):
    """Bresenham-spread n_act ScalarE ops among n_ops. True = ACT."""
    out = []
    acc = 0
    for _ in range(n_ops):
        acc += n_act
        if acc >= n_ops:
            acc -= n_ops
            out.append(True)
        else:
            out.append(False)
    assert sum(out) == n_act
    return out


def _dedup_ldweights(nc):
    """Drop InstLdweights whose weights match the previous LDW in the same
    basic block (the PE keeps the loaded stationary across matmuls)."""
    removed = 0
    for fn in nc.m.functions:
        for blk in fn.blocks:
            insts = list(blk.instructions)
            last_sig = None
            keep = []
            for inst in insts:
                if type(inst).__name__ == "InstLdweights":
                    si = inst.sync_info
                    clean = si is None or (
                        not list(si.on_wait) and not list(si.on_update))
                    sig = str(inst.ins[-1])
                    if sig == last_sig and clean:
                        removed += 1
                        continue
                    last_sig = sig
                keep.append(inst)
            if len(keep) != len(insts):
                blk.instructions = keep
    return removed


def _build_nc(n_blocks=N_BLOCKS, lcols=LCOLS, chunk=CHUNK, n_act=None,
              mm_free=MM_FREE, psum_bufs=PSUM_BUFS, dyn_loop=False,
              repeats=1, dedup_ldw=True, consumer="mixed", kaug=K_AUG,
              plan=PLAN):
    """Build + compile the per-core SPMD program.

    Inputs : rg_all [K_AUG, n_blocks*128] bf16, l_all [K_AUG, lcols] bf16
             (+ rep_cnt [1,1] i32 when dyn_loop)
    Outputs: dve_part [128, n_dve] f32 (max of bound-thr over chunk)
             act_part [128, n_act] f32 (sum of relu(bound-thr))
    """
    import concourse.tile as tile
    from concourse import bacc, mybir
    from concourse.bass import make_scalar_value, RegisterHandles

    if plan is not None:
        assert sum(fd for _, fd in plan) == lcols
        ops = list(plan) * n_blocks
    else:
        if n_act is None:
            n_act = N_ACT * (n_blocks * (lcols // chunk)) // N_OPS
        aa = _assignment(n_blocks * (lcols // chunk), n_act)
        ops = [("A" if a else "D", chunk) for a in aa]
    n_ops = len(ops)
    if consumer == "none":
        n_act = 0
        n_dve = 1
    else:
        n_act = sum(1 for e, _ in ops if e == "A")
        n_dve = n_ops - n_act
    ngc = n_blocks * 128

    nc = bacc.Bacc("TRN2", target_bir_lowering=False, debug=False,
                   num_devices=N_CORES)
    bf16 = mybir.dt.bfloat16
    f32 = mybir.dt.float32

    rg_in = nc.dram_tensor("rg_all", [kaug, ngc], bf16,
                           kind="ExternalInput").ap()
    l_in = nc.dram_tensor("l_all", [kaug, lcols], bf16,
                          kind="ExternalInput").ap()
    cnt_in = None
    if dyn_loop:
        cnt_in = nc.dram_tensor("rep_cnt", [1, 1], mybir.dt.int32,
                                kind="ExternalInput").ap()
    dve_out = act_out = None
    if n_dve:
        dve_out = nc.dram_tensor("dve_part", [128, n_dve], f32,
                                 kind="ExternalOutput").ap()
    if n_act:
        act_out = nc.dram_tensor("act_part", [128, n_act], f32,
                                 kind="ExternalOutput").ap()

    with tile.TileContext(nc) as tc:
        with (tc.tile_pool(name="io", bufs=1) as io_pool,
              tc.tile_pool(name="psum", bufs=(1 if plan is not None
                                              else psum_bufs),
                           space="PSUM") as psum_pool,
              tc.tile_pool(name="scratch", bufs=1) as scratch_pool):
            # tiny dummy activation first so the ACT table set loads during
            # the DMA prologue instead of before the first real op
            warm = scratch_pool.tile([128, 1], f32)
            nc.vector.memset(warm[:], 0.0)
            nc.scalar.activation(warm[:], warm[:],
                                 mybir.ActivationFunctionType.Relu)

            rg_sb = io_pool.tile([kaug, ngc], bf16)
            nc.sync.dma_start(rg_sb[:], rg_in[:])
            l_sb = io_pool.tile([kaug, lcols], bf16)
            # first chunk lands first so op 0 starts ASAP
            nc.sync.dma_start(l_sb[:, :chunk], l_in[:, :chunk])
            for c0 in range(chunk, lcols, chunk):
                nc.sync.dma_start(l_sb[:, c0:c0 + chunk],
                                  l_in[:, c0:c0 + chunk])

            dve_sb = act_sb = None
            if n_dve:
                dve_sb = io_pool.tile([128, n_dve], f32, name="dve_sb")
            if n_act:
                act_sb = io_pool.tile([128, n_act], f32, name="act_sb")
            if consumer == "none" and dve_sb is not None:
                nc.vector.memset(dve_sb[:], 0.0)

            if plan is not None:
                ps_all = psum_pool.tile([128, 4096], f32, name="ps_all")

            def body():
                dve_slot = 0
                act_slot = 0
                if plan is None:
                    # legacy path: rotating per-op pool tiles
                    op = 0
                    for b in range(n_blocks):
                        stat = rg_sb[:, b * 128:(b + 1) * 128]
                        for ch in range(lcols // chunk):
                            ps = psum_pool.tile([128, chunk], f32)
                            for k in range(chunk // mm_free):
                                ncol = ch * chunk + k * mm_free
                                nc.tensor.matmul(
                                    ps[:, k * mm_free:(k + 1) * mm_free],
                                    stat, l_sb[:, ncol:ncol + mm_free],
                                    start=True, stop=True)
                            if consumer == "none":
                                op += 1
                                continue
                            eng, _ = ops[op]
                            if eng == "A":
                                nc.scalar.activation(
                                    ps[:, :], ps[:, :],
                                    mybir.ActivationFunctionType.Relu,
                                    accum_out=act_sb[:, act_slot:act_slot + 1])
                                act_slot += 1
                            else:
                                nc.vector.tensor_reduce(
                                    dve_sb[:, dve_slot:dve_slot + 1],
                                    ps[:, :], axis=mybir.AxisListType.X,
                                    op=mybir.AluOpType.max)
                                dve_slot += 1
                            op += 1
                    return
                # plan path: one persistent PSUM tile, offsets cycle mod 4096
                for b in range(n_blocks):
                    stat = rg_sb[:, b * 128:(b + 1) * 128]
                    pos = 0
                    for eng, fd in plan:
                        o = pos % 4096
                        assert o + fd <= 4096
                        for k in range(0, fd, mm_free):
                            w = min(mm_free, fd - k)
                            nc.tensor.matmul(
                                ps_all[:, o + k:o + k + w],
                                stat, l_sb[:, pos + k:pos + k + w],
                                start=True, stop=True)
                        if consumer != "none":
                            if eng == "A":
                                nc.scalar.activation(
                                    ps_all[:, o:o + fd], ps_all[:, o:o + fd],
                                    mybir.ActivationFunctionType.Relu,
                                    accum_out=act_sb[:, act_slot:act_slot + 1])
                                act_slot += 1
                            else:
                                nc.vector.tensor_reduce(
                                    dve_sb[:, dve_slot:dve_slot + 1],
                                    ps_all[:, o:o + fd],
                                    axis=mybir.AxisListType.X,
                                    op=mybir.AluOpType.max)
                                dve_slot += 1
                        pos += fd

            if dyn_loop:
                cnt_sb = io_pool.tile([1, 1], mybir.dt.int32)
                nc.sync.dma_start(cnt_sb[:], cnt_in[:])
                regs = []
                for etype in mybir.ALL_ENGINES:
                    eng = nc.engines[etype]
                    reg = eng.alloc_register(f"repcnt_{etype.name}")
                    eng.reg_load(reg, cnt_sb[0:1, 0:1])
                    regs.append(reg)
                end_sv = make_scalar_value(
                    RegisterHandles(regs), min_val=0, max_val=100000)
                with tc.For_i(0, end_sv):
                    for _ in range(repeats):
                        body()
            else:
                for _ in range(repeats):
                    body()

            if dve_out is not None:
                nc.sync.dma_start(dve_out[:], dve_sb[:])
            if act_out is not None:
                nc.sync.dma_start(act_out[:], act_sb[:])

    if dedup_ldw:
        _dedup_ldweights(nc)
    nc.compile()
    return nc


def _get_nc(key=("full", 1)):
    if key not in _COMPILED:
        kind, repeats = key
        _COMPILED[key] = _build_nc(repeats=repeats)
    return _COMPILED[key]


# ---------------------------------------------------------------- host side

def _candidate_set(x64, xn, nrm_top=96, nproj=24, proj_top=8, seed=777):
    """Indices of extreme-norm / extreme-projection points."""
    cs = set(np.argsort(-xn)[:nrm_top].tolist())
    rng = np.random.default_rng(seed)
    U = rng.standard_normal((nproj, DIM))
    U /= np.linalg.norm(U, axis=1, keepdims=True)
    p = x64 @ U.T
    for k in range(nproj):
        cs.update(np.argsort(-p[:, k])[:proj_top].tolist())
        cs.update(np.argsort(p[:, k])[:proj_top].tolist())
    return np.array(sorted(cs))


def _est_heat(targets, cand_pts):
    """max_i d2(cand_i, target_j) for each target row (float64)."""
    tsq = (targets * targets).sum(1)
    csq = (cand_pts * cand_pts).sum(1)
    d2 = csq[:, None] + tsq[None, :] - 2.0 * (cand_pts @ targets.T)
    return d2.max(axis=0)


def _greedy_pair(pts):
    """Greedy min-distance matching of pts [n, d] -> [n//2, 2] local idx."""
    n = pts.shape[0]
    sq = (pts * pts).sum(1)
    d2 = sq[:, None] + sq[None, :] - 2.0 * (pts @ pts.T)
    np.fill_diagonal(d2, np.inf)
    k = min(8, n - 1)
    nbr = np.argpartition(d2, k - 1, axis=1)[:, :k]
    w = np.take_along_axis(d2, nbr, axis=1)
    edges = np.stack([np.repeat(np.arange(n), k), nbr.ravel(), w.ravel()], 1)
    edges = edges[np.argsort(edges[:, 2])]
    matched = np.zeros(n, dtype=bool)
    pairs = []
    for a, b, _ in edges:
        a, b = int(a), int(b)
        if not matched[a] and not matched[b]:
            matched[a] = matched[b] = True
            pairs.append((a, b))
    rest = np.nonzero(~matched)[0]
    while len(rest) > 1:
        subd = d2[np.ix_(rest, rest)]
        order = np.argsort(subd.ravel())
        used = np.zeros(len(rest), dtype=bool)
        for e in order:
            i, j = divmod(int(e), len(rest))
            if i != j and not used[i] and not used[j]:
                used[i] = used[j] = True
                pairs.append((int(rest[i]), int(rest[j])))
        rest = rest[~used]
    return np.array(pairs, dtype=np.int64)


def _cluster_bucket(r, idx, g):
    """Group r[idx] into size-g groups via hierarchical greedy pairing."""
    if g == 1:
        return [np.array([j]) for j in idx]
    cur = [np.array([j]) for j in idx]
    cents = r[idx].copy()
    while len(cur[0]) < g and len(cur) > 1:
        pairs = _greedy_pair(cents)
        newg, newc = [], []
        used = np.zeros(len(cur), dtype=bool)
        for a, b in pairs:
            na, nb = len(cur[a]), len(cur[b])
            newg.append(np.concatenate([cur[a], cur[b]]))
            newc.append((cents[a] * na + cents[b] * nb) / (na + nb))
            used[a] = used[b] = True
        for i in np.nonzero(~used)[0]:
            newg.append(cur[i])
            newc.append(cents[i])
        cur, cents = newg, np.array(newc)
    return cur


def _one_center(mem):
    """mem [ng, g, d] -> approx minimax centers [ng, d]."""
    c = mem.mean(axis=1)
    for t in range(25):
        d = np.sqrt(((mem - c[:, None, :]) ** 2).sum(-1))
        far = np.argmax(d, axis=1)
        fp = mem[np.arange(len(mem)), far]
        c = c + (1.0 / (t + 3)) * (fp - c)
    return c


def _candidate_threshold(l64, r64, lc, rc):
    """Exact (float64) max squared distance over the candidate pair set."""
    A = l64[lc]
    B = r64[rc]
    d2 = ((A * A).sum(1)[:, None] + (B * B).sum(1)[None, :]
          - 2.0 * (A @ B.T))
    return float(d2.max())


def _bf16_up(x):
    """Round x (f64 array) to bf16, forcing result >= x."""
    x = np.asarray(x, dtype=np.float64)
    y = x.astype(np.float32).astype(BF16)
    bad = y.astype(np.float64) < x
    if np.any(bad):
        bits = y.view(np.uint16)
        pos = (bits & 0x8000) == 0
        # next bf16 toward +inf: +1 ulp for positives, -1 for negatives
        # (negative zero / exact-zero handled via the pos mask on bits)
        up_bits = np.where(pos, bits + 1,
                           np.where(bits == 0x8000, np.uint16(0x0001),
                                    bits - 1)).astype(np.uint16)
        up = up_bits.view(BF16)
        y = np.where(bad, up, y)
        assert np.all(y.astype(np.float64) >= x)
    return y


def _prepare_all(l, r):
    """Returns (in_maps per core, meta dict)."""
    l64 = l.astype(np.float64)
    r64 = r.astype(np.float64)
    lsq = (l64 * l64).sum(1)
    rsq = (r64 * r64).sum(1)
    ln = np.sqrt(lsq)
    rn = np.sqrt(rsq)

    lc = _candidate_set(l64, ln)
    rc = _candidate_set(r64, rn)
    L = _candidate_threshold(l64, r64, lc, rc)

    # heat estimates for grouping / sorting (speed only, not correctness)
    m_est = _est_heat(r64, l64[lc])     # per r-point
    mu_est = _est_heat(l64, r64[rc])    # per l-row

    # group r-points: hottest get smallest groups
    order = np.argsort(m_est)[::-1]
    groups = []
    pos = 0
    for cnt, g in QUOTA:
        idx = order[pos:pos + cnt * g]
        pos += cnt * g
        groups.extend(_cluster_bucket(r64, idx, g))
    assert pos == N_R
    gsz = np.array([len(x) for x in groups])
    ng = len(groups)
    assert ng <= NG, (ng, NG)

    # group stats (vectorized per distinct size)
    cs = np.zeros((ng, DIM))
    rads = np.zeros(ng)
    ss = np.zeros(ng)
    for g in np.unique(gsz):
        sel = np.nonzero(gsz == g)[0]
        if g == 1:
            ids = [groups[i][0] for i in sel]
            cs[sel] = r64[ids]
            ss[sel] = rsq[ids]
            continue
        mem = np.stack([r64[groups[i]] for i in sel])
        c = _one_center(mem)
        cs[sel] = c
        rads[sel] = np.sqrt(((mem - c[:, None, :]) ** 2).sum(-1)).max(1)
        ss[sel] = np.stack([rsq[groups[i]] for i in sel]).max(1)

    # rigorous bf16/accum error bound for the cross term -2 l.c
    cn = np.sqrt((cs * cs).sum(1))
    delta = (2.0 ** -8) * 1.05 * (2.0 * ln.max() * max(cn.max(), 1e-9)) + 0.05
    thr = L - delta

    # device tensors.  bound slots rounded UP so device bound >= true bound
    l_aug = np.zeros((K_AUG, N_L), dtype=BF16)
    rg_aug = np.zeros((K_AUG, NG), dtype=BF16)

    # heat-sorted l (hot rows first -> flags concentrate in early chunks)
    lorder = np.argsort(mu_est)[::-1].copy()
    ls = l64[lorder]
    l_aug[:DIM] = ls.T.astype(np.float32).astype(BF16)
    l_aug[64] = _bf16_up(lsq[lorder])
    l_aug[65] = _bf16_up(ln[lorder])
    l_aug[66] = BF16(1.0)

    rg_aug[:DIM, :ng] = (-2.0 * cs.T).astype(np.float32).astype(BF16)
    rg_aug[64, :ng] = BF16(1.0)
    rg_aug[65, :ng] = _bf16_up(2.0 * rads)
    rg_aug[66, :ng] = _bf16_up(ss - thr)
    if ng < NG:  # padding groups: never flag
        rg_aug[66, ng:] = BF16(-1000.0)

    in_maps = [
        {"rg_all": np.ascontiguousarray(
            rg_aug[:, c * N_BLOCKS * 128:(c + 1) * N_BLOCKS * 128]),
         "l_all": np.ascontiguousarray(l_aug)}
        for c in range(N_CORES)
    ]
    meta = dict(groups=groups, gsz=gsz, ng=ng, L=L, thr=thr, delta=delta,
                lorder=lorder, lsq=lsq, rsq=rsq, l64=l64, r64=r64)
    return in_maps, meta


def _run_device(in_maps, nc=None):
    from concourse.bass_utils import run_bass_kernel_spmd
    if nc is None:
        nc = _get_nc()
    res = run_bass_kernel_spmd(nc, in_maps, core_ids=list(range(N_CORES)))
    return res.results


def _decode_and_recheck(results, meta):
    """Exact float64 recheck of flagged (group, l-chunk) cells."""
    groups = meta["groups"]
    lorder = meta["lorder"]
    lsq, rsq = meta["lsq"], meta["rsq"]
    l64, r64 = meta["l64"], meta["r64"]
    best = meta["L"]
    ng = meta["ng"]
    # op list: (engine, block, l-range)
    op_list = []
    for b in range(N_BLOCKS):
        pos = 0
        for eng, fd in _op_list():
            op_list.append((eng, b, pos, fd))
            pos += fd

    for core in range(N_CORES):
        dve = results[core].get("dve_part")
        act = results[core].get("act_part")
        dve_slot = act_slot = 0
        for eng, b, pos, fd in op_list:
            if eng == "A":
                part = act[:, act_slot]
                act_slot += 1
            else:
                part = dve[:, dve_slot]
                dve_slot += 1
            lanes = np.nonzero(part > 0.0)[0]
            if lanes.size == 0:
                continue
            rows = lorder[pos:pos + fd]
            members = []
            for p in lanes:
                gid = (core * N_BLOCKS + b) * 128 + int(p)
                if gid < ng:
                    members.append(groups[gid])
            if not members:
                continue
            mem = np.concatenate(members)
            d2 = (lsq[rows][:, None] + rsq[mem][None, :]
                  - 2.0 * (l64[rows] @ r64[mem].T))
            best = max(best, float(d2.max()))
    return best


def kernel(l_dfa_embeddings, r_dfa_embeddings):
    l = np.asarray(l_dfa_embeddings, dtype=np.float32)
    r = np.asarray(r_dfa_embeddings, dtype=np.float32)
    assert l.shape == (N_L, DIM) and r.shape == (N_R, DIM)

    in_maps, meta = _prepare_all(l, r)
    results = _run_device(in_maps)
    best = _decode_and_recheck(results, meta)
    return np.float32(np.sqrt(max(best, 0.0)))


# revision 29
# speedup vs baseline: 2.1409x; 1.0295x over previous
"""Max pairwise L2 distance between two embedding sets, on 8 Trainium2 cores.

Problem: l [8192, 64] f32, r [8192, 64] f32 -> scalar f32
    out = sqrt(max_ij ||l_i - r_j||^2)

Strategy (v2: group-bound certificate)
--------------------------------------
The distance matrix has 67M entries.  On TRN2 only VectorE/ScalarE can read
PSUM (1 fp32/lane/cycle), so exhaustive per-pair examination costs ~35us.
Instead each PSUM entry certifies a whole GROUP of r-points via a provable
upper bound.  For a group G with center c, radius rad = max_j ||r_j - c||,
s = max_j ||r_j||^2:

    max_{j in G} ||l_i - r_j||^2 <= lsq_i + s - 2 l_i.c + 2 ||l_i|| rad

which is a single dot product of augmented vectors:
    rg_vec = [-2c (64) | 1 | 2*rad | s - thr]      (stationary, bf16)
    l_vec  = [l_i (64) | lsq_i | ||l_i|| | 1]      (moving, bf16)

1. Host picks a candidate max L over extreme-norm/extreme-projection pairs
   and sets thr = L - delta (delta rigorously bounds bf16 rounding).
2. Host groups r-points by "heat" (estimated max distance to any l, via the
   candidate l-set): hot points get tiny groups (size 1-2, tight bound),
   the quiet bulk gets coarse groups (4-16).  NG total groups.
3. Device: r-group vectors are the stationary operand (NG/8 = blocks of 128
   per core), all 8192 l-columns stream as moving.  ScalarE relu+accum and
   VectorE max-reduce consume PSUM; a positive partial flags (group,
   l-chunk) cells.  l-rows are heat-sorted so flags concentrate in few
   cells.
4. Host rechecks flagged cells exactly (float64) and returns
   sqrt(max(L, flagged maxima)) - an exact fp32 answer for ANY input;
   grouping quality only affects speed.

Measured (8-core SPMD, per-pass device time): ~4.4-5.3us vs the 50-53us
v1 exhaustive kernel.  Key HW facts: the PE streams bf16 matmuls ~2.2x
slower when the contraction dim K < 128, so the augmented vectors are
zero-padded to K=128 (2.6us for 8192 moving cols at the production
roofline); PSUM consumption is the bottleneck (ScalarE (172+FD)/1.2GHz,
VectorE (120+FD)/0.96GHz per op), balanced 4 ACT + 4 DVE ops of FD=1024
over a 4-deep rotating PSUM pool.  Uneven ACT/DVE free dims via a single
persistent PSUM tile measured slower (overlap-dep chains serialize), and
PE-W + engine-R must never share a PSUM bank (fatal HW hazard), which
quantizes op regions to 512-f32 banks.
"""

import numpy as np
import ml_dtypes

N_CORES = 8
N_L, N_R, DIM = 8192, 8192, 64
K_AUG = 128                     # 64 dims + lsq/ln/1 rows + zero pad to 128
                                # (K<128 streams ~2.2x slower on the PE)
NG = 768                        # total r-groups (6 blocks of 128)
N_BLOCKS = 2                    # stationary blocks per core (2-job layout)
# Per-core jobs: (job-local stationary block, l-cols).  8 cores x job1
# (2048 cols) cover blocks B0,B1 in quarters; 8 cores x job2 (4096 cols)
# cover B2..B5 in halves.  Every lane stays filled with 6144 entries
# instead of 8192 (NG=1024), cutting PSUM consumption ~25%.
JOBS = ((0, 2048), (1, 4096))
LCOLS = sum(ncols for _, ncols in JOBS)
CHUNK = 1024                    # l-cols consumed per PSUM op
MM_FREE = 512                   # moving free dim per matmul (1 PSUM bank)
PSUM_BUFS = 4                   # rotating pool tiles (4 x 1024 = full PSUM)
# PLAN: optional explicit (engine, fd) op list per block for the
# persistent-tile path.  Measured slower than pool rotation (overlap-dep
# chains), so production uses PLAN=None -> uniform CHUNK ops, N_ACT of
# them on ScalarE (rest VectorE), Bresenham-interleaved.
PLAN = None
N_OPS = LCOLS // CHUNK
N_ACT = 3
# hottest r-points first: (count, group_size); counts*sizes must sum to N_R
QUOTA = ((232, 1), (28, 2), (28, 8), (480, 16))
BF16 = ml_dtypes.bfloat16

_COMPILED = {}


def _op_list():
    """Production op list per block: [(engine, fd), ...] covering LCOLS."""
    if PLAN is not None:
        return list(PLAN)
    aa = _assignment(LCOLS // CHUNK, N_ACT)
    return [("A" if a else "D", CHUNK) for a in aa]


def _assignment(n_ops=N_OPS, n_act=N_ACT):
    """Bresenham-spread n_act ScalarE ops among n_ops. True = ACT."""
    out = []
    acc = 0
    for _ in range(n_ops):
        acc += n_act
        if acc >= n_ops:
            acc -= n_ops
            out.append(True)
        else:
            out.append(False)
    assert sum(out) == n_act
    return out


def _dedup_ldweights(nc):
    """Drop InstLdweights whose weights match the previous LDW in the same
    basic block (the PE keeps the loaded stationary across matmuls)."""
    removed = 0
    for fn in nc.m.functions:
        for blk in fn.blocks:
            insts = list(blk.instructions)
            last_sig = None
            keep = []
            for inst in insts:
                if type(inst).__name__ == "InstLdweights":
                    si = inst.sync_info
                    clean = si is None or (
                        not list(si.on_wait) and not list(si.on_update))
                    sig = str(inst.ins[-1])
                    if sig == last_sig and clean:
                        removed += 1
                        continue
                    last_sig = sig
                keep.append(inst)
            if len(keep) != len(insts):
                blk.instructions = keep
    return removed


def _build_nc(n_blocks=N_BLOCKS, lcols=LCOLS, chunk=CHUNK, n_act=None,
              mm_free=MM_FREE, psum_bufs=PSUM_BUFS, dyn_loop=False,
              repeats=1, dedup_ldw=True, consumer="mixed", kaug=K_AUG,
              plan=PLAN):
    """Build + compile the per-core SPMD program.

    Inputs : rg_all [K_AUG, n_blocks*128] bf16, l_all [K_AUG, lcols] bf16
             (+ rep_cnt [1,1] i32 when dyn_loop)
    Outputs: dve_part [128, n_dve] f32 (max of bound-thr over chunk)
             act_part [128, n_act] f32 (sum of relu(bound-thr))
    """
    import concourse.tile as tile
    from concourse import bacc, mybir
    from concourse.bass import make_scalar_value, RegisterHandles

    if n_act is None:
        n_act = N_ACT
    aa = _assignment(lcols // chunk, n_act)
    ops = [("A" if a else "D", chunk) for a in aa]
    n_ops = len(ops)
    if consumer == "none":
        n_act = 0
        n_dve = 1
    else:
        n_act = sum(1 for e, _ in ops if e == "A")
        n_dve = n_ops - n_act
    ngc = n_blocks * 128

    nc = bacc.Bacc("TRN2", target_bir_lowering=False, debug=False,
                   num_devices=N_CORES)
    bf16 = mybir.dt.bfloat16
    f32 = mybir.dt.float32

    rg_in = nc.dram_tensor("rg_all", [kaug, ngc], bf16,
                           kind="ExternalInput").ap()
    l_in = nc.dram_tensor("l_all", [kaug, lcols], bf16,
                          kind="ExternalInput").ap()
    cnt_in = None
    if dyn_loop:
        cnt_in = nc.dram_tensor("rep_cnt", [1, 1], mybir.dt.int32,
                                kind="ExternalInput").ap()
    dve_out = act_out = None
    if n_dve:
        dve_out = nc.dram_tensor("dve_part", [128, n_dve], f32,
                                 kind="ExternalOutput").ap()
    if n_act:
        act_out = nc.dram_tensor("act_part", [128, n_act], f32,
                                 kind="ExternalOutput").ap()

    with tile.TileContext(nc) as tc:
        with (tc.tile_pool(name="io", bufs=1) as io_pool,
              tc.tile_pool(name="psum", bufs=psum_bufs,
                           space="PSUM") as psum_pool,
              tc.tile_pool(name="scratch", bufs=1) as scratch_pool):
            # tiny dummy activation first so the ACT table set loads during
            # the DMA prologue instead of before the first real op
            warm = scratch_pool.tile([128, 1], f32)
            nc.vector.memset(warm[:], 0.0)
            nc.scalar.activation(warm[:], warm[:],
                                 mybir.ActivationFunctionType.Relu)

            rg_sb = io_pool.tile([kaug, ngc], bf16)
            nc.sync.dma_start(rg_sb[:], rg_in[:])
            l_sb = io_pool.tile([kaug, lcols], bf16)
            # first chunk lands first so op 0 starts ASAP
            nc.sync.dma_start(l_sb[:, :chunk], l_in[:, :chunk])
            for c0 in range(chunk, lcols, chunk):
                nc.sync.dma_start(l_sb[:, c0:c0 + chunk],
                                  l_in[:, c0:c0 + chunk])

            dve_sb = act_sb = None
            if n_dve:
                dve_sb = io_pool.tile([128, n_dve], f32, name="dve_sb")
            if n_act:
                act_sb = io_pool.tile([128, n_act], f32, name="act_sb")
            if consumer == "none" and dve_sb is not None:
                nc.vector.memset(dve_sb[:], 0.0)

            def body():
                dve_slot = 0
                act_slot = 0
                op = 0
                pos = 0
                for jb, ncols in JOBS:
                    stat = rg_sb[:, jb * 128:(jb + 1) * 128]
                    for ch in range(ncols // chunk):
                        ps = psum_pool.tile([128, chunk], f32)
                        for k in range(chunk // mm_free):
                            ncol = pos + ch * chunk + k * mm_free
                            nc.tensor.matmul(
                                ps[:, k * mm_free:(k + 1) * mm_free],
                                stat, l_sb[:, ncol:ncol + mm_free],
                                start=True, stop=True)
                        if consumer == "none":
                            op += 1
                            continue
                        eng, _ = ops[op]
                        if eng == "A":
                            nc.scalar.activation(
                                ps[:, :], ps[:, :],
                                mybir.ActivationFunctionType.Relu,
                                accum_out=act_sb[:, act_slot:act_slot + 1])
                            act_slot += 1
                        else:
                            nc.vector.tensor_reduce(
                                dve_sb[:, dve_slot:dve_slot + 1],
                                ps[:, :], axis=mybir.AxisListType.X,
                                op=mybir.AluOpType.max)
                            dve_slot += 1
                        op += 1
                    pos += ncols

            if dyn_loop:
                cnt_sb = io_pool.tile([1, 1], mybir.dt.int32)
                nc.sync.dma_start(cnt_sb[:], cnt_in[:])
                regs = []
                for etype in mybir.ALL_ENGINES:
                    eng = nc.engines[etype]
                    reg = eng.alloc_register(f"repcnt_{etype.name}")
                    eng.reg_load(reg, cnt_sb[0:1, 0:1])
                    regs.append(reg)
                end_sv = make_scalar_value(
                    RegisterHandles(regs), min_val=0, max_val=100000)
                with tc.For_i(0, end_sv):
                    for _ in range(repeats):
                        body()
            else:
                for _ in range(repeats):
                    body()

            if dve_out is not None:
                nc.sync.dma_start(dve_out[:], dve_sb[:])
            if act_out is not None:
                nc.sync.dma_start(act_out[:], act_sb[:])

    if dedup_ldw:
        _dedup_ldweights(nc)
    nc.compile()
    return nc


def _get_nc(key=("full", 1)):
    if key not in _COMPILED:
        kind, repeats = key
        _COMPILED[key] = _build_nc(repeats=repeats)
    return _COMPILED[key]


# ---------------------------------------------------------------- host side

def _candidate_set(x64, xn, nrm_top=96, nproj=24, proj_top=8, seed=777):
    """Indices of extreme-norm / extreme-projection points."""
    cs = set(np.argsort(-xn)[:nrm_top].tolist())
    rng = np.random.default_rng(seed)
    U = rng.standard_normal((nproj, DIM))
    U /= np.linalg.norm(U, axis=1, keepdims=True)
    p = x64 @ U.T
    for k in range(nproj):
        cs.update(np.argsort(-p[:, k])[:proj_top].tolist())
        cs.update(np.argsort(p[:, k])[:proj_top].tolist())
    return np.array(sorted(cs))


def _est_heat(targets, cand_pts):
    """max_i d2(cand_i, target_j) for each target row (float64)."""
    tsq = (targets * targets).sum(1)
    csq = (cand_pts * cand_pts).sum(1)
    d2 = csq[:, None] + tsq[None, :] - 2.0 * (cand_pts @ targets.T)
    return d2.max(axis=0)


def _greedy_pair(pts):
    """Greedy min-distance matching of pts [n, d] -> [n//2, 2] local idx."""
    n = pts.shape[0]
    sq = (pts * pts).sum(1)
    d2 = sq[:, None] + sq[None, :] - 2.0 * (pts @ pts.T)
    np.fill_diagonal(d2, np.inf)
    k = min(8, n - 1)
    nbr = np.argpartition(d2, k - 1, axis=1)[:, :k]
    w = np.take_along_axis(d2, nbr, axis=1)
    edges = np.stack([np.repeat(np.arange(n), k), nbr.ravel(), w.ravel()], 1)
    edges = edges[np.argsort(edges[:, 2])]
    matched = np.zeros(n, dtype=bool)
    pairs = []
    for a, b, _ in edges:
        a, b = int(a), int(b)
        if not matched[a] and not matched[b]:
            matched[a] = matched[b] = True
            pairs.append((a, b))
    rest = np.nonzero(~matched)[0]
    while len(rest) > 1:
        subd = d2[np.ix_(rest, rest)]
        order = np.argsort(subd.ravel())
        used = np.zeros(len(rest), dtype=bool)
        for e in order:
            i, j = divmod(int(e), len(rest))
            if i != j and not used[i] and not used[j]:
                used[i] = used[j] = True
                pairs.append((int(rest[i]), int(rest[j])))
        rest = rest[~used]
    return np.array(pairs, dtype=np.int64)


def _cluster_bucket(r, idx, g):
    """Group r[idx] into size-g groups via hierarchical greedy pairing."""
    if g == 1:
        return [np.array([j]) for j in idx]
    cur = [np.array([j]) for j in idx]
    cents = r[idx].copy()
    while len(cur[0]) < g and len(cur) > 1:
        pairs = _greedy_pair(cents)
        newg, newc = [], []
        used = np.zeros(len(cur), dtype=bool)
        for a, b in pairs:
            na, nb = len(cur[a]), len(cur[b])
            newg.append(np.concatenate([cur[a], cur[b]]))
            newc.append((cents[a] * na + cents[b] * nb) / (na + nb))
            used[a] = used[b] = True
        for i in np.nonzero(~used)[0]:
            newg.append(cur[i])
            newc.append(cents[i])
        cur, cents = newg, np.array(newc)
    return cur


def _one_center(mem):
    """mem [ng, g, d] -> approx minimax centers [ng, d]."""
    c = mem.mean(axis=1)
    for t in range(25):
        d = np.sqrt(((mem - c[:, None, :]) ** 2).sum(-1))
        far = np.argmax(d, axis=1)
        fp = mem[np.arange(len(mem)), far]
        c = c + (1.0 / (t + 3)) * (fp - c)
    return c


def _candidate_threshold(l64, r64, lc, rc):
    """Exact (float64) max squared distance over the candidate pair set."""
    A = l64[lc]
    B = r64[rc]
    d2 = ((A * A).sum(1)[:, None] + (B * B).sum(1)[None, :]
          - 2.0 * (A @ B.T))
    return float(d2.max())


def _bf16_up(x):
    """Round x (f64 array) to bf16, forcing result >= x."""
    x = np.asarray(x, dtype=np.float64)
    y = x.astype(np.float32).astype(BF16)
    bad = y.astype(np.float64) < x
    if np.any(bad):
        bits = y.view(np.uint16)
        pos = (bits & 0x8000) == 0
        # next bf16 toward +inf: +1 ulp for positives, -1 for negatives
        # (negative zero / exact-zero handled via the pos mask on bits)
        up_bits = np.where(pos, bits + 1,
                           np.where(bits == 0x8000, np.uint16(0x0001),
                                    bits - 1)).astype(np.uint16)
        up = up_bits.view(BF16)
        y = np.where(bad, up, y)
        assert np.all(y.astype(np.float64) >= x)
    return y


def _prepare_all(l, r):
    """Returns (in_maps per core, meta dict)."""
    l64 = l.astype(np.float64)
    r64 = r.astype(np.float64)
    lsq = (l64 * l64).sum(1)
    rsq = (r64 * r64).sum(1)
    ln = np.sqrt(lsq)
    rn = np.sqrt(rsq)

    lc = _candidate_set(l64, ln)
    rc = _candidate_set(r64, rn)
    L = _candidate_threshold(l64, r64, lc, rc)

    # heat estimates for grouping / sorting (speed only, not correctness)
    m_est = _est_heat(r64, l64[lc])     # per r-point
    mu_est = _est_heat(l64, r64[rc])    # per l-row

    # group r-points: hottest get smallest groups
    order = np.argsort(m_est)[::-1]
    groups = []
    pos = 0
    for cnt, g in QUOTA:
        idx = order[pos:pos + cnt * g]
        pos += cnt * g
        groups.extend(_cluster_bucket(r64, idx, g))
    assert pos == N_R
    gsz = np.array([len(x) for x in groups])
    ng = len(groups)
    assert ng <= NG, (ng, NG)

    # group stats (vectorized per distinct size)
    cs = np.zeros((ng, DIM))
    rads = np.zeros(ng)
    ss = np.zeros(ng)
    for g in np.unique(gsz):
        sel = np.nonzero(gsz == g)[0]
        if g == 1:
            ids = [groups[i][0] for i in sel]
            cs[sel] = r64[ids]
            ss[sel] = rsq[ids]
            continue
        mem = np.stack([r64[groups[i]] for i in sel])
        c = _one_center(mem)
        cs[sel] = c
        rads[sel] = np.sqrt(((mem - c[:, None, :]) ** 2).sum(-1)).max(1)
        ss[sel] = np.stack([rsq[groups[i]] for i in sel]).max(1)

    # rigorous bf16/accum error bound for the cross term -2 l.c
    cn = np.sqrt((cs * cs).sum(1))
    delta = (2.0 ** -8) * 1.05 * (2.0 * ln.max() * max(cn.max(), 1e-9)) + 0.05
    thr = L - delta

    # device tensors.  bound slots rounded UP so device bound >= true bound
    l_aug = np.zeros((K_AUG, N_L), dtype=BF16)
    rg_aug = np.zeros((K_AUG, NG), dtype=BF16)

    # heat-sorted l (hot rows first -> flags concentrate in early chunks)
    lorder = np.argsort(mu_est)[::-1].copy()
    ls = l64[lorder]
    l_aug[:DIM] = ls.T.astype(np.float32).astype(BF16)
    l_aug[64] = _bf16_up(lsq[lorder])
    l_aug[65] = _bf16_up(ln[lorder])
    l_aug[66] = BF16(1.0)

    rg_aug[:DIM, :ng] = (-2.0 * cs.T).astype(np.float32).astype(BF16)
    rg_aug[64, :ng] = BF16(1.0)
    rg_aug[65, :ng] = _bf16_up(2.0 * rads)
    rg_aug[66, :ng] = _bf16_up(ss - thr)
    if ng < NG:  # padding groups: never flag
        rg_aug[66, ng:] = BF16(-1000.0)

    # per-core 2-job layout: job1 (2048 cols) covers blocks B0/B1 in
    # quarters; job2 (4096 cols) covers B2..B5 in halves.  core_jobs[c] =
    # [(gid_base, l_lo, ncols), ...] in device op order.
    in_maps = []
    core_jobs = []
    for c in range(N_CORES):
        b1 = c // 4                      # B0 or B1
        q = c % 4
        b2 = 2 + c // 2                  # B2..B5
        h = c % 2
        jobs = [(b1 * 128, q * 2048, 2048),
                (b2 * 128, h * 4096, 4096)]
        core_jobs.append(jobs)
        rg_core = np.concatenate(
            [rg_aug[:, b1 * 128:(b1 + 1) * 128],
             rg_aug[:, b2 * 128:(b2 + 1) * 128]], axis=1)
        l_core = np.concatenate(
            [l_aug[:, jobs[0][1]:jobs[0][1] + jobs[0][2]],
             l_aug[:, jobs[1][1]:jobs[1][1] + jobs[1][2]]], axis=1)
        in_maps.append({"rg_all": np.ascontiguousarray(rg_core),
                        "l_all": np.ascontiguousarray(l_core)})
    meta = dict(groups=groups, gsz=gsz, ng=ng, L=L, thr=thr, delta=delta,
                lorder=lorder, lsq=lsq, rsq=rsq, l64=l64, r64=r64,
                core_jobs=core_jobs)
    return in_maps, meta


def _run_device(in_maps, nc=None):
    from concourse.bass_utils import run_bass_kernel_spmd
    if nc is None:
        nc = _get_nc()
    res = run_bass_kernel_spmd(nc, in_maps, core_ids=list(range(N_CORES)))
    return res.results


def _decode_and_recheck(results, meta):
    """Exact float64 recheck of flagged (group, l-chunk) cells."""
    groups = meta["groups"]
    lorder = meta["lorder"]
    lsq, rsq = meta["lsq"], meta["rsq"]
    l64, r64 = meta["l64"], meta["r64"]
    best = meta["L"]
    ng = meta["ng"]
    core_jobs = meta["core_jobs"]
    engines = _op_list()

    for core in range(N_CORES):
        dve = results[core].get("dve_part")
        act = results[core].get("act_part")
        dve_slot = act_slot = 0
        # op order: jobs in sequence, CHUNK-sized ops within each job
        op_descs = []
        for gid_base, l_lo, ncols in core_jobs[core]:
            for ch in range(ncols // CHUNK):
                op_descs.append((gid_base, l_lo + ch * CHUNK))
        assert len(op_descs) == len(engines)
        for (eng, fd), (gid_base, pos) in zip(engines, op_descs):
            if eng == "A":
                part = act[:, act_slot]
                act_slot += 1
            else:
                part = dve[:, dve_slot]
                dve_slot += 1
            lanes = np.nonzero(part > 0.0)[0]
            if lanes.size == 0:
                continue
            rows = lorder[pos:pos + fd]
            members = []
            for p in lanes:
                gid = gid_base + int(p)
                if gid < ng:
                    members.append(groups[gid])
            if not members:
                continue
            mem = np.concatenate(members)
            d2 = (lsq[rows][:, None] + rsq[mem][None, :]
                  - 2.0 * (l64[rows] @ r64[mem].T))
            best = max(best, float(d2.max()))
    return best


def kernel(l_dfa_embeddings, r_dfa_embeddings):
    l = np.asarray(l_dfa_embeddings, dtype=np.float32)
    r = np.asarray(r_dfa_embeddings, dtype=np.float32)
    assert l.shape == (N_L, DIM) and r.shape == (N_R, DIM)

    in_maps, meta = _prepare_all(l, r)
    results = _run_device(in_maps)
    best = _decode_and_recheck(results, meta)
    return np.float32(np.sqrt(max(best, 0.0)))


# revision 31
# speedup vs baseline: 2.2293x; 1.0413x over previous
"""Max pairwise L2 distance between two embedding sets, on 8 Trainium2 cores.

Problem: l [8192, 64] f32, r [8192, 64] f32 -> scalar f32
    out = sqrt(max_ij ||l_i - r_j||^2)

Strategy (v2: group-bound certificate)
--------------------------------------
The distance matrix has 67M entries.  On TRN2 only VectorE/ScalarE can read
PSUM (1 fp32/lane/cycle), so exhaustive per-pair examination costs ~35us.
Instead each PSUM entry certifies a whole GROUP of r-points via a provable
upper bound.  For a group G with center c, radius rad = max_j ||r_j - c||,
s = max_j ||r_j||^2:

    max_{j in G} ||l_i - r_j||^2 <= lsq_i + s - 2 l_i.c + 2 ||l_i|| rad

which is a single dot product of augmented vectors:
    rg_vec = [-2c (64) | 1 | 2*rad | s - thr]      (stationary, bf16)
    l_vec  = [l_i (64) | lsq_i | ||l_i|| | 1]      (moving, bf16)

1. Host picks a candidate max L over extreme-norm/extreme-projection pairs
   and sets thr = L - delta (delta rigorously bounds bf16 rounding).
2. Host groups r-points by "heat" (estimated max distance to any l, via the
   candidate l-set): hot points get tiny groups (size 1-2, tight bound),
   the quiet bulk gets coarse groups (4-16).  NG total groups.
3. Device: r-group vectors are the stationary operand (NG/8 = blocks of 128
   per core), all 8192 l-columns stream as moving.  ScalarE relu+accum and
   VectorE max-reduce consume PSUM; a positive partial flags (group,
   l-chunk) cells.  l-rows are heat-sorted so flags concentrate in few
   cells.
4. Host rechecks flagged cells exactly (float64) and returns
   sqrt(max(L, flagged maxima)) - an exact fp32 answer for ANY input;
   grouping quality only affects speed.

Measured (8-core SPMD, per-pass device time): ~3.7-4.5us vs the 50-53us
v1 exhaustive kernel.  Key HW facts: the PE streams bf16 matmuls ~2.2x
slower when the contraction dim K < 128, so the augmented vectors are
zero-padded to K=128 (at the production roofline); PSUM consumption is
the bottleneck (ScalarE (172+FD)/1.2GHz, VectorE (120+FD)/0.96GHz per
op, 1 fp32/lane/cycle), so group count is the lever: NG=768 groups in a
2-job-per-core layout (job1: blocks B0/B1 x l-quarters, job2: B2..B5 x
l-halves) keeps all 128 lanes full at 6144 entries/lane, consumed by
3 ACT + 3 DVE ops of FD=1024 over a 4-deep rotating PSUM pool.  Uneven
ACT/DVE free dims via a single persistent PSUM tile measured slower
(overlap-dep chains serialize), and PE-W + engine-R must never share a
PSUM bank (fatal HW hazard), which quantizes op regions to 512-f32
banks.
"""

import numpy as np
import ml_dtypes

N_CORES = 8
N_L, N_R, DIM = 8192, 8192, 64
K_AUG = 128                     # 64 dims + lsq/ln/1 rows + zero pad to 128
                                # (K<128 streams ~2.2x slower on the PE)
NG = 768                        # total r-groups (6 blocks of 128)
N_BLOCKS = 2                    # stationary blocks per core (2-job layout)
# Per-core jobs: (job-local stationary block, l-cols).  8 cores x job1
# (2048 cols) cover blocks B0,B1 in quarters; 8 cores x job2 (4096 cols)
# cover B2..B5 in halves.  Every lane stays filled with 6144 entries
# instead of 8192 (NG=1024), cutting PSUM consumption ~25%.
JOBS = ((0, 2048), (1, 4096))
LCOLS = sum(ncols for _, ncols in JOBS)
CHUNK = 1024                    # l-cols consumed per PSUM op
MM_FREE = 256                   # moving free dim per matmul (half a PSUM
                                # bank; measured 22% faster per pass than
                                # 512 — finer PE/consumer interleave)
PSUM_BUFS = 4                   # rotating pool tiles (4 x 1024 = full PSUM)
# PLAN: optional explicit (engine, fd) op list per block for the
# persistent-tile path.  Measured slower than pool rotation (overlap-dep
# chains), so production uses PLAN=None -> uniform CHUNK ops, N_ACT of
# them on ScalarE (rest VectorE), Bresenham-interleaved.
PLAN = None
N_OPS = LCOLS // CHUNK
N_ACT = 3
# hottest r-points first: (count, group_size); counts*sizes must sum to N_R
QUOTA = ((232, 1), (28, 2), (28, 8), (480, 16))
BF16 = ml_dtypes.bfloat16

_COMPILED = {}


def _op_list():
    """Production op list per block: [(engine, fd), ...] covering LCOLS."""
    if PLAN is not None:
        return list(PLAN)
    aa = _assignment(LCOLS // CHUNK, N_ACT)
    return [("A" if a else "D", CHUNK) for a in aa]


def _assignment(n_ops=N_OPS, n_act=N_ACT):
    """Bresenham-spread n_act ScalarE ops among n_ops. True = ACT."""
    out = []
    acc = 0
    for _ in range(n_ops):
        acc += n_act
        if acc >= n_ops:
            acc -= n_ops
            out.append(True)
        else:
            out.append(False)
    assert sum(out) == n_act
    return out


def _dedup_ldweights(nc):
    """Drop InstLdweights whose weights match the previous LDW in the same
    basic block (the PE keeps the loaded stationary across matmuls)."""
    removed = 0
    for fn in nc.m.functions:
        for blk in fn.blocks:
            insts = list(blk.instructions)
            last_sig = None
            keep = []
            for inst in insts:
                if type(inst).__name__ == "InstLdweights":
                    si = inst.sync_info
                    clean = si is None or (
                        not list(si.on_wait) and not list(si.on_update))
                    sig = str(inst.ins[-1])
                    if sig == last_sig and clean:
                        removed += 1
                        continue
                    last_sig = sig
                keep.append(inst)
            if len(keep) != len(insts):
                blk.instructions = keep
    return removed


def _build_nc(n_blocks=N_BLOCKS, lcols=LCOLS, chunk=CHUNK, n_act=None,
              mm_free=MM_FREE, psum_bufs=PSUM_BUFS, dyn_loop=False,
              repeats=1, dedup_ldw=True, consumer="mixed", kaug=K_AUG,
              plan=PLAN):
    """Build + compile the per-core SPMD program.

    Inputs : rg_all [K_AUG, n_blocks*128] bf16, l_all [K_AUG, lcols] bf16
             (+ rep_cnt [1,1] i32 when dyn_loop)
    Outputs: dve_part [128, n_dve] f32 (max of bound-thr over chunk)
             act_part [128, n_act] f32 (sum of relu(bound-thr))
    """
    import concourse.tile as tile
    from concourse import bacc, mybir
    from concourse.bass import make_scalar_value, RegisterHandles

    if n_act is None:
        n_act = N_ACT
    aa = _assignment(lcols // chunk, n_act)
    ops = [("A" if a else "D", chunk) for a in aa]
    n_ops = len(ops)
    if consumer == "none":
        n_act = 0
        n_dve = 1
    else:
        n_act = sum(1 for e, _ in ops if e == "A")
        n_dve = n_ops - n_act
    ngc = n_blocks * 128

    nc = bacc.Bacc("TRN2", target_bir_lowering=False, debug=False,
                   num_devices=N_CORES)
    bf16 = mybir.dt.bfloat16
    f32 = mybir.dt.float32

    rg_in = nc.dram_tensor("rg_all", [kaug, ngc], bf16,
                           kind="ExternalInput").ap()
    l_in = nc.dram_tensor("l_all", [kaug, lcols], bf16,
                          kind="ExternalInput").ap()
    cnt_in = None
    if dyn_loop:
        cnt_in = nc.dram_tensor("rep_cnt", [1, 1], mybir.dt.int32,
                                kind="ExternalInput").ap()
    dve_out = act_out = None
    if n_dve:
        dve_out = nc.dram_tensor("dve_part", [128, n_dve], f32,
                                 kind="ExternalOutput").ap()
    if n_act:
        act_out = nc.dram_tensor("act_part", [128, n_act], f32,
                                 kind="ExternalOutput").ap()

    with tile.TileContext(nc) as tc:
        with (tc.tile_pool(name="io", bufs=1) as io_pool,
              tc.tile_pool(name="psum", bufs=psum_bufs,
                           space="PSUM") as psum_pool,
              tc.tile_pool(name="scratch", bufs=1) as scratch_pool):
            # tiny dummy activation first so the ACT table set loads during
            # the DMA prologue instead of before the first real op
            warm = scratch_pool.tile([128, 1], f32)
            nc.vector.memset(warm[:], 0.0)
            nc.scalar.activation(warm[:], warm[:],
                                 mybir.ActivationFunctionType.Relu)

            rg_sb = io_pool.tile([kaug, ngc], bf16)
            nc.sync.dma_start(rg_sb[:], rg_in[:])
            l_sb = io_pool.tile([kaug, lcols], bf16)
            # first chunk lands first so op 0 starts ASAP
            nc.sync.dma_start(l_sb[:, :chunk], l_in[:, :chunk])
            for c0 in range(chunk, lcols, chunk):
                nc.sync.dma_start(l_sb[:, c0:c0 + chunk],
                                  l_in[:, c0:c0 + chunk])

            dve_sb = act_sb = None
            if n_dve:
                dve_sb = io_pool.tile([128, n_dve], f32, name="dve_sb")
            if n_act:
                act_sb = io_pool.tile([128, n_act], f32, name="act_sb")
            if consumer == "none" and dve_sb is not None:
                nc.vector.memset(dve_sb[:], 0.0)

            def body():
                dve_slot = 0
                act_slot = 0
                op = 0
                pos = 0
                for jb, ncols in JOBS:
                    stat = rg_sb[:, jb * 128:(jb + 1) * 128]
                    for ch in range(ncols // chunk):
                        ps = psum_pool.tile([128, chunk], f32)
                        for k in range(chunk // mm_free):
                            ncol = pos + ch * chunk + k * mm_free
                            nc.tensor.matmul(
                                ps[:, k * mm_free:(k + 1) * mm_free],
                                stat, l_sb[:, ncol:ncol + mm_free],
                                start=True, stop=True)
                        if consumer == "none":
                            op += 1
                            continue
                        eng, _ = ops[op]
                        if eng == "A":
                            nc.scalar.activation(
                                ps[:, :], ps[:, :],
                                mybir.ActivationFunctionType.Relu,
                                accum_out=act_sb[:, act_slot:act_slot + 1])
                            act_slot += 1
                        else:
                            nc.vector.tensor_reduce(
                                dve_sb[:, dve_slot:dve_slot + 1],
                                ps[:, :], axis=mybir.AxisListType.X,
                                op=mybir.AluOpType.max)
                            dve_slot += 1
                        op += 1
                    pos += ncols

            if dyn_loop:
                cnt_sb = io_pool.tile([1, 1], mybir.dt.int32)
                nc.sync.dma_start(cnt_sb[:], cnt_in[:])
                regs = []
                for etype in mybir.ALL_ENGINES:
                    eng = nc.engines[etype]
                    reg = eng.alloc_register(f"repcnt_{etype.name}")
                    eng.reg_load(reg, cnt_sb[0:1, 0:1])
                    regs.append(reg)
                end_sv = make_scalar_value(
                    RegisterHandles(regs), min_val=0, max_val=100000)
                with tc.For_i(0, end_sv):
                    for _ in range(repeats):
                        body()
            else:
                for _ in range(repeats):
                    body()

            if dve_out is not None:
                nc.sync.dma_start(dve_out[:], dve_sb[:])
            if act_out is not None:
                nc.sync.dma_start(act_out[:], act_sb[:])

    if dedup_ldw:
        _dedup_ldweights(nc)
    nc.compile()
    return nc


def _get_nc(key=("full", 1)):
    if key not in _COMPILED:
        kind, repeats = key
        _COMPILED[key] = _build_nc(repeats=repeats)
    return _COMPILED[key]


# ---------------------------------------------------------------- host side

def _candidate_set(x64, xn, nrm_top=96, nproj=24, proj_top=8, seed=777):
    """Indices of extreme-norm / extreme-projection points."""
    cs = set(np.argsort(-xn)[:nrm_top].tolist())
    rng = np.random.default_rng(seed)
    U = rng.standard_normal((nproj, DIM))
    U /= np.linalg.norm(U, axis=1, keepdims=True)
    p = x64 @ U.T
    for k in range(nproj):
        cs.update(np.argsort(-p[:, k])[:proj_top].tolist())
        cs.update(np.argsort(p[:, k])[:proj_top].tolist())
    return np.array(sorted(cs))


def _est_heat(targets, cand_pts):
    """max_i d2(cand_i, target_j) for each target row (float64)."""
    tsq = (targets * targets).sum(1)
    csq = (cand_pts * cand_pts).sum(1)
    d2 = csq[:, None] + tsq[None, :] - 2.0 * (cand_pts @ targets.T)
    return d2.max(axis=0)


def _greedy_pair(pts):
    """Greedy min-distance matching of pts [n, d] -> [n//2, 2] local idx."""
    n = pts.shape[0]
    sq = (pts * pts).sum(1)
    d2 = sq[:, None] + sq[None, :] - 2.0 * (pts @ pts.T)
    np.fill_diagonal(d2, np.inf)
    k = min(8, n - 1)
    nbr = np.argpartition(d2, k - 1, axis=1)[:, :k]
    w = np.take_along_axis(d2, nbr, axis=1)
    edges = np.stack([np.repeat(np.arange(n), k), nbr.ravel(), w.ravel()], 1)
    edges = edges[np.argsort(edges[:, 2])]
    matched = np.zeros(n, dtype=bool)
    pairs = []
    for a, b, _ in edges:
        a, b = int(a), int(b)
        if not matched[a] and not matched[b]:
            matched[a] = matched[b] = True
            pairs.append((a, b))
    rest = np.nonzero(~matched)[0]
    while len(rest) > 1:
        subd = d2[np.ix_(rest, rest)]
        order = np.argsort(subd.ravel())
        used = np.zeros(len(rest), dtype=bool)
        for e in order:
            i, j = divmod(int(e), len(rest))
            if i != j and not used[i] and not used[j]:
                used[i] = used[j] = True
                pairs.append((int(rest[i]), int(rest[j])))
        rest = rest[~used]
    return np.array(pairs, dtype=np.int64)


def _cluster_bucket(r, idx, g):
    """Group r[idx] into size-g groups via hierarchical greedy pairing."""
    if g == 1:
        return [np.array([j]) for j in idx]
    cur = [np.array([j]) for j in idx]
    cents = r[idx].copy()
    while len(cur[0]) < g and len(cur) > 1:
        pairs = _greedy_pair(cents)
        newg, newc = [], []
        used = np.zeros(len(cur), dtype=bool)
        for a, b in pairs:
            na, nb = len(cur[a]), len(cur[b])
            newg.append(np.concatenate([cur[a], cur[b]]))
            newc.append((cents[a] * na + cents[b] * nb) / (na + nb))
            used[a] = used[b] = True
        for i in np.nonzero(~used)[0]:
            newg.append(cur[i])
            newc.append(cents[i])
        cur, cents = newg, np.array(newc)
    return cur


def _one_center(mem):
    """mem [ng, g, d] -> approx minimax centers [ng, d]."""
    c = mem.mean(axis=1)
    for t in range(25):
        d = np.sqrt(((mem - c[:, None, :]) ** 2).sum(-1))
        far = np.argmax(d, axis=1)
        fp = mem[np.arange(len(mem)), far]
        c = c + (1.0 / (t + 3)) * (fp - c)
    return c


def _candidate_threshold(l64, r64, lc, rc):
    """Exact (float64) max squared distance over the candidate pair set."""
    A = l64[lc]
    B = r64[rc]
    d2 = ((A * A).sum(1)[:, None] + (B * B).sum(1)[None, :]
          - 2.0 * (A @ B.T))
    return float(d2.max())


def _bf16_up(x):
    """Round x (f64 array) to bf16, forcing result >= x."""
    x = np.asarray(x, dtype=np.float64)
    y = x.astype(np.float32).astype(BF16)
    bad = y.astype(np.float64) < x
    if np.any(bad):
        bits = y.view(np.uint16)
        pos = (bits & 0x8000) == 0
        # next bf16 toward +inf: +1 ulp for positives, -1 for negatives
        # (negative zero / exact-zero handled via the pos mask on bits)
        up_bits = np.where(pos, bits + 1,
                           np.where(bits == 0x8000, np.uint16(0x0001),
                                    bits - 1)).astype(np.uint16)
        up = up_bits.view(BF16)
        y = np.where(bad, up, y)
        assert np.all(y.astype(np.float64) >= x)
    return y


def _prepare_all(l, r):
    """Returns (in_maps per core, meta dict)."""
    l64 = l.astype(np.float64)
    r64 = r.astype(np.float64)
    lsq = (l64 * l64).sum(1)
    rsq = (r64 * r64).sum(1)
    ln = np.sqrt(lsq)
    rn = np.sqrt(rsq)

    lc = _candidate_set(l64, ln)
    rc = _candidate_set(r64, rn)
    L = _candidate_threshold(l64, r64, lc, rc)

    # heat estimates for grouping / sorting (speed only, not correctness)
    m_est = _est_heat(r64, l64[lc])     # per r-point
    mu_est = _est_heat(l64, r64[rc])    # per l-row

    # group r-points: hottest get smallest groups
    order = np.argsort(m_est)[::-1]
    groups = []
    pos = 0
    for cnt, g in QUOTA:
        idx = order[pos:pos + cnt * g]
        pos += cnt * g
        groups.extend(_cluster_bucket(r64, idx, g))
    assert pos == N_R
    gsz = np.array([len(x) for x in groups])
    ng = len(groups)
    assert ng <= NG, (ng, NG)

    # group stats (vectorized per distinct size)
    cs = np.zeros((ng, DIM))
    rads = np.zeros(ng)
    ss = np.zeros(ng)
    for g in np.unique(gsz):
        sel = np.nonzero(gsz == g)[0]
        if g == 1:
            ids = [groups[i][0] for i in sel]
            cs[sel] = r64[ids]
            ss[sel] = rsq[ids]
            continue
        mem = np.stack([r64[groups[i]] for i in sel])
        c = _one_center(mem)
        cs[sel] = c
        rads[sel] = np.sqrt(((mem - c[:, None, :]) ** 2).sum(-1)).max(1)
        ss[sel] = np.stack([rsq[groups[i]] for i in sel]).max(1)

    # rigorous bf16/accum error bound for the cross term -2 l.c
    cn = np.sqrt((cs * cs).sum(1))
    delta = (2.0 ** -8) * 1.05 * (2.0 * ln.max() * max(cn.max(), 1e-9)) + 0.05
    thr = L - delta

    # device tensors.  bound slots rounded UP so device bound >= true bound
    l_aug = np.zeros((K_AUG, N_L), dtype=BF16)
    rg_aug = np.zeros((K_AUG, NG), dtype=BF16)

    # heat-sorted l (hot rows first -> flags concentrate in early chunks)
    lorder = np.argsort(mu_est)[::-1].copy()
    ls = l64[lorder]
    l_aug[:DIM] = ls.T.astype(np.float32).astype(BF16)
    l_aug[64] = _bf16_up(lsq[lorder])
    l_aug[65] = _bf16_up(ln[lorder])
    l_aug[66] = BF16(1.0)

    rg_aug[:DIM, :ng] = (-2.0 * cs.T).astype(np.float32).astype(BF16)
    rg_aug[64, :ng] = BF16(1.0)
    rg_aug[65, :ng] = _bf16_up(2.0 * rads)
    rg_aug[66, :ng] = _bf16_up(ss - thr)
    if ng < NG:  # padding groups: never flag
        rg_aug[66, ng:] = BF16(-1000.0)

    # per-core 2-job layout: job1 (2048 cols) covers blocks B0/B1 in
    # quarters; job2 (4096 cols) covers B2..B5 in halves.  core_jobs[c] =
    # [(gid_base, l_lo, ncols), ...] in device op order.
    in_maps = []
    core_jobs = []
    for c in range(N_CORES):
        b1 = c // 4                      # B0 or B1
        q = c % 4
        b2 = 2 + c // 2                  # B2..B5
        h = c % 2
        jobs = [(b1 * 128, q * 2048, 2048),
                (b2 * 128, h * 4096, 4096)]
        core_jobs.append(jobs)
        rg_core = np.concatenate(
            [rg_aug[:, b1 * 128:(b1 + 1) * 128],
             rg_aug[:, b2 * 128:(b2 + 1) * 128]], axis=1)
        l_core = np.concatenate(
            [l_aug[:, jobs[0][1]:jobs[0][1] + jobs[0][2]],
             l_aug[:, jobs[1][1]:jobs[1][1] + jobs[1][2]]], axis=1)
        in_maps.append({"rg_all": np.ascontiguousarray(rg_core),
                        "l_all": np.ascontiguousarray(l_core)})
    meta = dict(groups=groups, gsz=gsz, ng=ng, L=L, thr=thr, delta=delta,
                lorder=lorder, lsq=lsq, rsq=rsq, l64=l64, r64=r64,
                core_jobs=core_jobs)
    return in_maps, meta


def _run_device(in_maps, nc=None):
    from concourse.bass_utils import run_bass_kernel_spmd
    if nc is None:
        nc = _get_nc()
    res = run_bass_kernel_spmd(nc, in_maps, core_ids=list(range(N_CORES)))
    return res.results


def _decode_and_recheck(results, meta):
    """Exact float64 recheck of flagged (group, l-chunk) cells."""
    groups = meta["groups"]
    lorder = meta["lorder"]
    lsq, rsq = meta["lsq"], meta["rsq"]
    l64, r64 = meta["l64"], meta["r64"]
    best = meta["L"]
    ng = meta["ng"]
    core_jobs = meta["core_jobs"]
    engines = _op_list()

    for core in range(N_CORES):
        dve = results[core].get("dve_part")
        act = results[core].get("act_part")
        dve_slot = act_slot = 0
        # op order: jobs in sequence, CHUNK-sized ops within each job
        op_descs = []
        for gid_base, l_lo, ncols in core_jobs[core]:
            for ch in range(ncols // CHUNK):
                op_descs.append((gid_base, l_lo + ch * CHUNK))
        assert len(op_descs) == len(engines)
        for (eng, fd), (gid_base, pos) in zip(engines, op_descs):
            if eng == "A":
                part = act[:, act_slot]
                act_slot += 1
            else:
                part = dve[:, dve_slot]
                dve_slot += 1
            lanes = np.nonzero(part > 0.0)[0]
            if lanes.size == 0:
                continue
            rows = lorder[pos:pos + fd]
            members = []
            for p in lanes:
                gid = gid_base + int(p)
                if gid < ng:
                    members.append(groups[gid])
            if not members:
                continue
            mem = np.concatenate(members)
            d2 = (lsq[rows][:, None] + rsq[mem][None, :]
                  - 2.0 * (l64[rows] @ r64[mem].T))
            best = max(best, float(d2.max()))
    return best


def kernel(l_dfa_embeddings, r_dfa_embeddings):
    l = np.asarray(l_dfa_embeddings, dtype=np.float32)
    r = np.asarray(r_dfa_embeddings, dtype=np.float32)
    assert l.shape == (N_L, DIM) and r.shape == (N_R, DIM)

    in_maps, meta = _prepare_all(l, r)
    results = _run_device(in_maps)
    best = _decode_and_recheck(results, meta)
    return np.float32(np.sqrt(max(best, 0.0)))
